# revision 45
# baseline (speedup 1.0000x reference)
"""Trainium2 Bass kernel for nn_NlsqCond (ConvFlow NLSQ coupling layer).

Strategy: pure data parallel over batch B=256 -> 32 samples per core on 8
NeuronCores. Convs are computed as 3 shifted matmuls accumulating in PSUM
over a gap-column activation layout (34 columns per sample, zero guard
columns), so the k=3/pad=1 conv needs no boundary special-casing. Weights
are transposed host-side into lhsT layout and cast to bf16 (fp32 PSUM
accumulation); measured end-to-end error vs fp32 reference is ~1e-4.
The final projection is computed transposed ([cols, 10]) so the NLSQ
elementwise tail runs with full 128-partition parallelism; the per-sample
logdet partition-reduction is done with a small mask matmul.
"""

import math

import numpy as np

B, T, D, H, COND = 256, 64, 2, 512, 8
T2 = T // 2                      # 32
NCORES = 8
NB = B // NCORES                 # 32 samples per core
ST = T2 + 2                      # 34: per-sample column stride (zero gaps)
WCOLS = NB * ST                  # 1088
NG = 2                           # PSUM column groups per matmul set
GS = NB // NG                    # 16 samples per group
NCHUNK = NB * T2 // 128          # 8 column chunks of 128 for final proj
LOG_A = math.log(8.0 * math.sqrt(3.0) / 9.0 - 0.05)

_CACHE = {}


def _build_program():
    import concourse.bacc as bacc
    import concourse.mybir as mybir
    import concourse.tile as tile

    f32 = mybir.dt.float32
    bf16 = mybir.dt.bfloat16
    AF = mybir.ActivationFunctionType
    Alu = mybir.AluOpType

    nc = bacc.Bacc("TRN2", target_bir_lowering=False, debug=False)

    # ---- DRAM I/O ----
    # z1 in gap layout (zero guard columns), rows replicated for the 3 conv
    # shifts: row (k*2+d) col m = z1_gap[d, m+k]. Feeds conv0's h-half as a
    # single K=6 matmul per window.
    d_z1g = nc.dram_tensor("z1g", [6, WCOLS], bf16, kind="ExternalInput")
    d_z2r = nc.dram_tensor("z2r", [128, NCHUNK, 2], f32, kind="ExternalInput")
    d_condT = nc.dram_tensor("condT", [2 * COND, NB], bf16, kind="ExternalInput")
    d_wc1 = nc.dram_tensor("wc1", [2 * COND, H], bf16, kind="ExternalInput")
    # conv/linear weights packed partition-major so each DMA moves one large
    # contiguous chunk per partition (descriptor-rate, not bandwidth, limits
    # small-row DMAs)
    d_wc2 = nc.dram_tensor("wc2", [128, 4, H], bf16, kind="ExternalInput")
    # conv0 h-half collapsed to rank 6: wh6[k*2+d] = (w_conv0[:, :H, k] @ we).T
    d_wh6 = nc.dram_tensor("wh6", [6, H], bf16, kind="ExternalInput")
    # conv0 cond-half collapsed to per-sample vectors: types (sum_k, k=0, k=2)
    d_qw = nc.dram_tensor("qw", [128, 4, 3, H], bf16, kind="ExternalInput")
    # b_embed fold rank-1 terms per type
    d_vb = nc.dram_tensor("vb", [1, 3, H], bf16, kind="ExternalInput")
    # signed indicator matrix applying q to the conv0 windows on the PE:
    # row ty*32+s, col m -> coefficient of q[ty][s] in output position m+1
    d_ik = nc.dram_tensor("ik", [96, WCOLS], bf16, kind="ExternalInput")
    d_wcv = nc.dram_tensor("wcv", [4, 128, 4, 3, H], bf16, kind="ExternalInput")
    d_wout = nc.dram_tensor("wout", [128, 4, 10], bf16, kind="ExternalInput")
    d_bias = nc.dram_tensor("biases", [128, 8, 4], f32, kind="ExternalInput")
    d_brep = nc.dram_tensor("brep", [128, NCHUNK, 10], f32, kind="ExternalInput")
    d_mask = nc.dram_tensor("mask", [128, 4], f32, kind="ExternalInput")
    d_z2n = nc.dram_tensor("z2n", [128, NCHUNK, 2], f32, kind="ExternalOutput")
    d_ld = nc.dram_tensor("ld", [NCHUNK, 4], f32, kind="ExternalOutput")

    with tile.TileContext(nc) as tc:
        with (
            tc.tile_pool(name="w", bufs=1) as wp,
            tc.tile_pool(name="act", bufs=1) as ap_,
            tc.tile_pool(name="ew", bufs=1) as ewp,
            tc.tile_pool(name="ps", bufs=8, space="PSUM") as pp,
        ):
            def mk(pool, shape, dtype, tag):
                return pool.tile(shape, dtype, tag=tag, name=tag)

            # ---- SBUF loads ----
            # issue DMAs from several engine queues in parallel (each
            # DMA_DIRECT2D costs ~0.6-1.3us of serial issue time per queue)
            z1g_sb = mk(ap_, [6, WCOLS], bf16, "z1g")
            nc.gpsimd.dma_start(z1g_sb[:], d_z1g.ap())
            wh6_sb = mk(wp, [6, H], bf16, "wh6")
            nc.gpsimd.dma_start(wh6_sb[:], d_wh6.ap())
            condT_sb = mk(ap_, [2 * COND, NB], bf16, "condT")
            nc.gpsimd.dma_start(condT_sb[:], d_condT.ap())
            wc1_sb = mk(wp, [2 * COND, H], bf16, "wc1")
            nc.gpsimd.dma_start(wc1_sb[:], d_wc1.ap())
            bias_sb = mk(wp, [128, 8, 4], f32, "bias")
            nc.gpsimd.dma_start(bias_sb[:], d_bias.ap())
            wc2_sb = mk(wp, [128, 4, H], bf16, "wc2")
            nc.scalar.dma_start(wc2_sb[:], d_wc2.ap())
            vb_sb = mk(wp, [1, 3, H], bf16, "vb")
            nc.scalar.dma_start(vb_sb[:], d_vb.ap())
            ik_sb = mk(wp, [96, WCOLS], bf16, "ik")
            nc.scalar.dma_start(ik_sb[:], d_ik.ap())
            qw_sb = mk(wp, [128, 4, 3, H], bf16, "qw")
            for kt in range(4):
                nc.scalar.dma_start(qw_sb[:, kt], d_qw.ap()[:, kt])
            wcv_sb = [mk(wp, [128, 4, 3, H], bf16, f"wcv_{l}") for l in range(4)]
            for l in range(4):
                nc.sync.dma_start(wcv_sb[l][:, 0:2], d_wcv.ap()[l, :, 0:2])
                nc.sync.dma_start(wcv_sb[l][:, 2:4], d_wcv.ap()[l, :, 2:4])
            wout_sb = mk(wp, [128, 4, 10], bf16, "wout")
            nc.scalar.dma_start(wout_sb[:], d_wout.ap())
            brep_sb = mk(wp, [128, NCHUNK, 10], f32, "brep")
            nc.scalar.dma_start(brep_sb[:], d_brep.ap())
            mask_sb = mk(wp, [128, 4], f32, "mask")
            nc.scalar.dma_start(mask_sb[:], d_mask.ap())
            z2r_sb = mk(ewp, [128, NCHUNK, 2], f32, "z2r")
            nc.scalar.dma_start(z2r_sb[:], d_z2r.ap())

            def bias_ap(idx, mt):
                return bias_sb[:, idx, mt : mt + 1]

            # ---- PE warm-up scratch (memset first so the PE can spin ASAP) ----
            wu_l = mk(ap_, [128, 128], bf16, "wu_l")
            wu_r = mk(ap_, [128, 512], bf16, "wu_r")
            nc.vector.memset(wu_l[:], 0.0)
            nc.vector.memset(wu_r[:], 0.0)

            # ---- activation ping-pong buffers (gap layout) ----
            # Only the gap guard columns need zeroing; valid columns are
            # always written before they are read.
            actB = [mk(ap_, [128, WCOLS], bf16, f"B{i}") for i in range(4)]
            actC = [mk(ap_, [128, WCOLS], bf16, f"C{i}") for i in range(4)]
            for ti, t_ in enumerate(actB + actC):
                v = t_[:].rearrange("p (s c) -> p s c", c=ST)
                eng = nc.vector if ti % 2 else nc.gpsimd
                eng.memset(v[:, :, 0:1], 0.0)
                eng.memset(v[:, :, ST - 1 : ST], 0.0)
            ones_sb = mk(ap_, [1, NB], bf16, "ones")
            nc.vector.memset(ones_sb[:], 1.0)

            def valid(tl, g=None):
                v = tl[:].rearrange("p (s c) -> p s c", c=ST)
                if g is None:
                    return v[:, :, 1 : 1 + T2]
                return v[:, g * GS : (g + 1) * GS, 1 : 1 + T2]

            # ---- PE warm-up spin ----
            # The PE HAM clock gate starts at 1.2 GHz and only releases to
            # 2.4 GHz after ~3.4us of sustained activity. Matmul on scratch
            # zeros while the weight DMAs stream in, so the real conv stack
            # runs warm from its first instruction.
            ps_w = mk(pp, [128, 512], f32, "ps")
            for i in range(16):
                nc.tensor.matmul(
                    ps_w[:], wu_l[:], wu_r[:], start=(i == 0), stop=(i == 15)
                )

            # preload the ACT transcendental tables so the elementwise tail
            # doesn't pay the table-swap latency
            scr = mk(ewp, [1, 4], f32, "scr")
            nc.scalar.activation(scr[:, 0:1], wu_l[:1, 0:1], AF.Exp)
            nc.scalar.activation(scr[:, 1:2], wu_l[:1, 0:1], AF.Tanh)
            nc.scalar.activation(scr[:, 2:3], wu_l[:1, 0:1], AF.Ln, bias=1.0)

            # ---- cond MLP: c2 = relu(W2 relu(W1 c + b1) + b2) ----
            c1_sb = [mk(ap_, [128, NB], bf16, f"c1_{i}") for i in range(4)]
            c2_sb = [mk(ap_, [128, NB], bf16, f"c2_{i}") for i in range(4)]
            for mt in range(4):
                ps = mk(pp, [128, 512], f32, "ps")
                nc.tensor.matmul(
                    ps[:, :NB],
                    wc1_sb[:, mt * 128 : (mt + 1) * 128],
                    condT_sb[:],
                    start=True,
                    stop=True,
                )
                nc.scalar.activation(
                    c1_sb[mt][:], ps[:, :NB], AF.Relu, bias=bias_ap(1, mt)
                )
            for mt in range(4):
                ps = mk(pp, [128, 512], f32, "ps")
                for kt in range(4):
                    nc.tensor.matmul(
                        ps[:, :NB],
                        wc2_sb[:, kt, mt * 128 : (mt + 1) * 128],
                        c1_sb[kt][:],
                        start=(kt == 0),
                        stop=(kt == 3),
                    )
                nc.scalar.activation(
                    c2_sb[mt][:], ps[:, :NB], AF.Relu, bias=bias_ap(2, mt)
                )

            WINDOWS = [(0, 15), (15, 15), (30, 2)]  # (sample base, n samples)

            def relu_copy(L, mt, wi, pv_, dsts, sb, ns):
                dv_ = dsts[mt][:].rearrange("p (s c) -> p s c", c=ST)[
                    :, sb : sb + ns, 1 : 1 + T2
                ]
                # split the PSUM->SBUF relu copies across ACT and DVE
                if (mt * 3 + wi) % 2 == 0:
                    nc.scalar.activation(dv_, pv_, AF.Relu, bias=bias_ap(3 + L, mt))
                else:
                    nc.vector.tensor_scalar(
                        dv_, pv_, bias_ap(3 + L, mt), 0.0, Alu.add, Alu.max
                    )

            # ---- conv0 ----
            # qT[ty*32+s, o]: per-sample cond/bias vectors, computed
            # transposed so they can be applied to the conv0 windows by a
            # single K=96 indicator matmul inside the PSUM accumulation:
            #   ty=0: (sum_k W0k_c) @ c2[s] + sum_k(W0k_h @ b_embed)
            #   ty=1: W00_c @ c2[s] + v0   (subtracted at t=0)
            #   ty=2: W02_c @ c2[s] + v2   (subtracted at t=31)
            psq = mk(pp, [96, 512], f32, "ps")
            for ty in range(3):
                for kt in range(4):
                    nc.tensor.matmul(
                        psq[ty * NB : (ty + 1) * NB, :],
                        c2_sb[kt][:],
                        qw_sb[:, kt, ty, :],
                        start=(kt == 0),
                        stop=False,
                    )
                nc.tensor.matmul(
                    psq[ty * NB : (ty + 1) * NB, :],
                    ones_sb[:],
                    vb_sb[:, ty, :],
                    start=False,
                    stop=True,
                )
            qT_sb = mk(ap_, [96, H], bf16, "qT")
            nc.vector.tensor_copy(qT_sb[:], psq[:])

            # h-half: single K=6 matmul per window (the 3 conv shifts are
            # baked into the replicated z1g rows); q applied via indicator
            for mt in range(4):
                pss = [mk(pp, [128, 512], f32, "ps") for _ in WINDOWS]
                for wi, (sb, ns) in enumerate(WINDOWS):
                    n = ns * ST - 2
                    base = sb * ST
                    nc.tensor.matmul(
                        pss[wi][:, :n],
                        wh6_sb[:, mt * 128 : (mt + 1) * 128],
                        z1g_sb[:, base : base + n],
                        start=True,
                        stop=False,
                    )
                    nc.tensor.matmul(
                        pss[wi][:, :n],
                        qT_sb[:, mt * 128 : (mt + 1) * 128],
                        ik_sb[:, base : base + n],
                        start=False,
                        stop=True,
                    )
                for wi, (sb, ns) in enumerate(WINDOWS):
                    pv3 = pss[wi][:, : ns * ST].rearrange("p (s c) -> p s c", c=ST)
                    relu_copy(0, mt, wi, pv3[:, :, 0:T2], actB, sb, ns)

            # ---- conv1..4 ----
            # Matmul moving operands must be single-free-dim, so each conv
            # matmul streams a contiguous window of the gap layout; outputs
            # at gap positions are garbage and simply never read back.
            for L in range(1, 5):
                srcs = [actB, actC, actB, actC][L - 1]
                wts = wcv_sb[L - 1][:]
                dsts = actB if L % 2 == 0 else actC
                for mt in range(4):
                    pss = [mk(pp, [128, 512], f32, "ps") for _ in WINDOWS]
                    nacc = 12
                    i = 0
                    # kt outer: matches weight-DMA arrival order
                    for kt in range(4):
                        for k in range(3):
                            lhsT = wts[:, kt, k, mt * 128 : (mt + 1) * 128]
                            for wi, (sb, ns) in enumerate(WINDOWS):
                                n = ns * ST - 2
                                base = sb * ST + k
                                nc.tensor.matmul(
                                    pss[wi][:, :n],
                                    lhsT,
                                    srcs[kt][:, base : base + n],
                                    start=(i == 0),
                                    stop=(i == nacc - 1),
                                )
                            i += 1
                    for wi, (sb, ns) in enumerate(WINDOWS):
                        pv3 = pss[wi][:, : ns * ST].rearrange(
                            "p (s c) -> p s c", c=ST
                        )
                        relu_copy(L, mt, wi, pv3[:, :, 0:T2], dsts, sb, ns)
            srcs = actB

            # ---- compact the final activation (drop gap columns) ----
            h5 = [mk(ap_, [128, NB * T2], bf16, f"h5_{i}") for i in range(4)]
            for kt in range(4):
                eng = nc.scalar if kt % 2 == 0 else nc.vector
                if kt % 2 == 0:
                    nc.scalar.copy(
                        h5[kt][:].rearrange("p (s t) -> p s t", t=T2),
                        valid(srcs[kt]),
                    )
                else:
                    nc.vector.tensor_copy(
                        h5[kt][:].rearrange("p (s t) -> p s t", t=T2),
                        valid(srcs[kt]),
                    )

            # ---- final projection (transposed): out[col, 10] ----
            pso = mk(pp, [128, 512], f32, "ps")
            for j in range(NCHUNK):
                for kt in range(4):
                    nc.tensor.matmul(
                        pso[:, j * 10 : (j + 1) * 10],
                        h5[kt][:, j * 128 : (j + 1) * 128],
                        wout_sb[:, kt],
                        start=(kt == 0),
                        stop=(kt == 3),
                    )

            # ---- NLSQ elementwise tail ----
            def ew(tag):
                return mk(ewp, [128, NCHUNK, 2], f32, tag)

            params = mk(ewp, [128, NCHUNK, 10], f32, "params")
            nc.vector.tensor_add(
                params[:],
                pso[:, : NCHUNK * 10].rearrange("p (j q) -> p j q", q=10),
                brep_sb[:],
            )
            pv = params[:].rearrange("p j (a q) -> p j a q", q=5)
            P0, P1, P2, P3, P4 = (pv[:, :, :, i] for i in range(5))

            loga_sb = mk(ewp, [128, 1], f32, "loga")
            nc.vector.memset(loga_sb[:], LOG_A)

            # group ACT functions (Exp x3, then Tanh, Ln last) to minimize
            # activation-table reloads
            u = ew("u")
            nc.vector.tensor_tensor(u[:], P1, P3, op=Alu.subtract)
            b_ = ew("b_")
            nc.scalar.activation(b_[:], P1, AF.Exp, scale=0.4)
            d_ = ew("d_")
            nc.scalar.activation(d_[:], P3, AF.Exp, scale=0.4)
            e = ew("e")
            nc.scalar.activation(e[:], u[:], AF.Exp, scale=0.4, bias=loga_sb[:])
            th = ew("th")
            nc.scalar.activation(th[:], P2, AF.Tanh, scale=0.3)
            c_ = ew("c_")
            nc.vector.tensor_mul(c_[:], th[:], e[:])
            t1 = ew("t1")
            nc.vector.tensor_mul(t1[:], d_[:], z2r_sb[:])
            arg = ew("arg")
            nc.vector.tensor_add(arg[:], t1[:], P4)
            sq = ew("sq")
            nc.vector.tensor_mul(sq[:], arg[:], arg[:])
            den = ew("den")
            nc.vector.tensor_scalar_add(den[:], sq[:], 1.0)
            rcp = ew("rcp")
            nc.vector.reciprocal(rcp[:], den[:])
            t2 = ew("t2")
            nc.vector.tensor_mul(t2[:], b_[:], z2r_sb[:])
            t3 = ew("t3")
            nc.vector.tensor_mul(t3[:], c_[:], rcp[:])
            s1 = ew("s1")
            nc.vector.tensor_add(s1[:], P0, t2[:])
            z2n_sb = ew("z2n_sb")
            nc.vector.tensor_add(z2n_sb[:], s1[:], t3[:])
            nc.sync.dma_start(d_z2n.ap(), z2n_sb[:])

            t4 = ew("t4")
            nc.vector.tensor_mul(t4[:], c_[:], d_[:])
            t5 = ew("t5")
            nc.vector.tensor_mul(t5[:], t4[:], arg[:])
            t6 = ew("t6")
            nc.vector.tensor_mul(t6[:], t5[:], rcp[:])
            t7 = ew("t7")
            nc.vector.tensor_mul(t7[:], t6[:], rcp[:])
            inner = ew("inner")
            nc.vector.scalar_tensor_tensor(
                inner[:], t7[:], -2.0, b_[:], op0=Alu.mult, op1=Alu.add
            )
            lg = ew("lg")
            nc.scalar.activation(lg[:], inner[:], AF.Ln)

            lg2 = mk(ewp, [128, NCHUNK], f32, "lg2")
            nc.vector.tensor_add(lg2[:], lg[:, :, 0], lg[:, :, 1])
            psl = mk(pp, [128, 512], f32, "ps")
            nc.tensor.matmul(
                psl[:NCHUNK, :4], lg2[:], mask_sb[:], start=True, stop=True
            )
            ld_sb = mk(ewp, [NCHUNK, 4], f32, "ld_sb")
            nc.vector.tensor_copy(ld_sb[:], psl[:NCHUNK, :4])
            nc.sync.dma_start(d_ld.ap(), ld_sb[:])

    nc.compile()
    return nc


def _get_program():
    if "nc" not in _CACHE:
        _CACHE["nc"] = _build_program()
    return _CACHE["nc"]


def _host_inputs(inputs):
    import ml_dtypes

    bf16 = ml_dtypes.bfloat16
    f32 = np.float32

    x = np.asarray(inputs["x"], f32)
    cond = np.asarray(inputs["cond"], f32)

    wc1T = np.ascontiguousarray(inputs["w_c1"].T).astype(bf16)
    # partition-major packing: [128, ...] with large contiguous per-partition
    # chunks so SBUF DMAs are few big descriptors per partition
    wc2T = np.ascontiguousarray(
        inputs["w_c2"].T.reshape(4, 128, H).transpose(1, 0, 2)
    ).astype(bf16)
    # conv0 h-half collapsed through the rank-2 embedding; rows (k*2+d)
    w0 = np.asarray(inputs["w_conv0"], f32)
    we2 = np.asarray(inputs["w_embed"], f32)[:, :2]
    wh6 = np.ascontiguousarray(
        np.einsum("ock,cd->kdo", w0[:, :H, :], we2).reshape(6, H)
    ).astype(bf16)
    # conv0 cond-half collapsed to per-sample vectors (types: sum_k, k0, k2)
    S = w0[:, H:, :]
    mats = np.stack([(S[:, :, 0] + S[:, :, 1] + S[:, :, 2]).T, S[:, :, 0].T, S[:, :, 2].T])
    qw = np.ascontiguousarray(
        mats.reshape(3, 4, 128, H).transpose(2, 1, 0, 3)
    ).astype(bf16)
    # b_embed folded through conv0 (rank-1 terms per type)
    be = np.asarray(inputs["b_embed"], f32)
    v = np.stack([w0[:, :H, k] @ be for k in range(3)])  # [3, H(out)]
    vb = np.ascontiguousarray(
        np.stack([v[0] + v[1] + v[2], v[0], v[2]])[None, :, :]
    ).astype(bf16)
    wcv = np.stack(
        [
            np.ascontiguousarray(
                np.transpose(inputs[f"w_conv{i}"], (1, 2, 0))
                .reshape(4, 128, 3, H)
                .transpose(1, 0, 2, 3)
            )
            for i in (1, 2, 3, 4)
        ]
    ).astype(bf16)
    woutT = np.ascontiguousarray(
        inputs["w_out"].T.reshape(4, 128, 10).transpose(1, 0, 2)
    ).astype(bf16)
    bias_all = np.stack(
        [inputs["b_embed"], inputs["b_c1"], inputs["b_c2"]]
        + [inputs[f"b_conv{i}"] for i in range(5)]
    ).astype(f32)
    bias_pack = np.ascontiguousarray(
        bias_all.reshape(8, 4, 128).transpose(2, 0, 1)
    ).astype(f32)
    brep = np.ascontiguousarray(
        np.broadcast_to(inputs["b_out"].astype(f32), (128, NCHUNK, 10))
    )
    mask = np.zeros((128, 4), f32)
    mask[np.arange(128), np.arange(128) // 32] = 1.0
    ik = np.zeros((96, WCOLS), f32)
    for s in range(NB):
        ik[s, ST * s : ST * s + T2] = 1.0
        ik[NB + s, ST * s] = -1.0
        ik[2 * NB + s, ST * s + T2 - 1] = -1.0
    ik = ik.astype(bf16)

    in_maps = []
    for c in range(NCORES):
        xs = x[c * NB : (c + 1) * NB]
        z1 = xs[:, :T2]
        z2 = xs[:, T2:]
        z1p = np.zeros((2, WCOLS + 2), np.float32)
        z1p[:, :WCOLS].reshape(2, NB, ST)[:, :, 1 : 1 + T2] = z1.transpose(2, 0, 1)
        z1g = np.stack(
            [z1p[dd, k : k + WCOLS] for k in range(3) for dd in range(2)]
        ).astype(bf16)
        z2r = np.ascontiguousarray(
            z2.reshape(NCHUNK, 4, T2, 2).transpose(1, 2, 0, 3)
        ).reshape(128, NCHUNK, 2)
        condT = np.ascontiguousarray(
            cond[c * NB : (c + 1) * NB].reshape(NB, 2 * COND).T
        ).astype(bf16)
        in_maps.append(
            dict(
                z1g=z1g,
                z2r=z2r,
                condT=condT,
                wc1=wc1T,
                wc2=wc2T,
                wh6=wh6,
                qw=qw,
                vb=vb,
                ik=ik,
                wcv=wcv,
                wout=woutT,
                biases=bias_pack,
                brep=brep,
                mask=mask,
            )
        )
    return in_maps


def _assemble_output(x, results):
    z = np.empty((B, T, D), np.float32)
    ld = np.empty((B,), np.float32)
    for c in range(NCORES):
        z[c * NB : (c + 1) * NB, :T2] = x[c * NB : (c + 1) * NB, :T2]
        z2n = np.asarray(results[c]["z2n"], np.float32)
        z[c * NB : (c + 1) * NB, T2:] = (
            z2n.reshape(4, T2, NCHUNK, 2).transpose(2, 0, 1, 3).reshape(NB, T2, 2)
        )
        ld[c * NB : (c + 1) * NB] = np.asarray(results[c]["ld"], np.float32).reshape(
            NB
        )
    return z, ld


def run(inputs, trace=False, trace_cores=None):
    """Run on 8 NeuronCores; returns ((z, logdet), BassKernelResults)."""
    from concourse.bass_utils import run_bass_kernel_spmd

    nc = _get_program()
    in_maps = _host_inputs(inputs)
    res = run_bass_kernel_spmd(
        nc,
        in_maps,
        list(range(NCORES)),
        trace=trace,
        trace_cores=trace_cores if trace_cores is not None else list(range(NCORES)),
    )
    x = np.asarray(inputs["x"], np.float32)
    return _assemble_output(x, res.results), res


def kernel(**inputs):
    (z, ld), _ = run(inputs, trace=False)
    return z, ld


if __name__ == "__main__":
    print("build only:", _get_program())


# revision 46
# speedup vs baseline: 1.0660x; 1.0660x over previous
"""Trainium2 Bass kernel for nn_NlsqCond (ConvFlow NLSQ coupling layer).

Strategy: pure data parallel over batch B=256 -> 32 samples per core on 8
NeuronCores. Convs are computed as 3 shifted matmuls accumulating in PSUM
over a gap-column activation layout (34 columns per sample, zero guard
columns), so the k=3/pad=1 conv needs no boundary special-casing. Weights
are transposed host-side into lhsT layout and cast to bf16 (fp32 PSUM
accumulation); measured end-to-end error vs fp32 reference is ~1e-4.
The final projection is computed transposed ([cols, 10]) so the NLSQ
elementwise tail runs with full 128-partition parallelism; the per-sample
logdet partition-reduction is done with a small mask matmul.
"""

import math

import numpy as np

B, T, D, H, COND = 256, 64, 2, 512, 8
T2 = T // 2                      # 32
NCORES = 8
NB = B // NCORES                 # 32 samples per core
ST = T2 + 2                      # 34: per-sample column stride (zero gaps)
WCOLS = NB * ST                  # 1088
NG = 2                           # PSUM column groups per matmul set
GS = NB // NG                    # 16 samples per group
NCHUNK = NB * T2 // 128          # 8 column chunks of 128 for final proj
LOG_A = math.log(8.0 * math.sqrt(3.0) / 9.0 - 0.05)

_CACHE = {}


def _build_program():
    import concourse.bacc as bacc
    import concourse.mybir as mybir
    import concourse.tile as tile

    f32 = mybir.dt.float32
    bf16 = mybir.dt.bfloat16
    AF = mybir.ActivationFunctionType
    Alu = mybir.AluOpType

    nc = bacc.Bacc("TRN2", target_bir_lowering=False, debug=False)

    # ---- DRAM I/O ----
    # z1 in gap layout (zero guard columns), rows replicated for the 3 conv
    # shifts: row (k*2+d) col m = z1_gap[d, m+k]. Feeds conv0's h-half as a
    # single K=6 matmul per window.
    d_z1g = nc.dram_tensor("z1g", [6, WCOLS], bf16, kind="ExternalInput")
    d_z2r = nc.dram_tensor("z2r", [128, NCHUNK, 2], f32, kind="ExternalInput")
    d_condT = nc.dram_tensor("condT", [2 * COND, NB], bf16, kind="ExternalInput")
    d_wc1 = nc.dram_tensor("wc1", [2 * COND, H], bf16, kind="ExternalInput")
    # conv/linear weights packed partition-major so each DMA moves one large
    # contiguous chunk per partition (descriptor-rate, not bandwidth, limits
    # small-row DMAs)
    d_wc2 = nc.dram_tensor("wc2", [128, 4, H], bf16, kind="ExternalInput")
    # conv0 h-half collapsed to rank 6: wh6[k*2+d] = (w_conv0[:, :H, k] @ we).T
    d_wh6 = nc.dram_tensor("wh6", [6, H], bf16, kind="ExternalInput")
    # conv0 cond-half collapsed to per-sample vectors: types (sum_k, k=0, k=2)
    d_qw = nc.dram_tensor("qw", [128, 4, 3, H], bf16, kind="ExternalInput")
    # b_embed fold rank-1 terms per type
    d_vb = nc.dram_tensor("vb", [1, 3, H], bf16, kind="ExternalInput")
    # signed indicator matrix applying q to the conv0 windows on the PE:
    # row ty*32+s, col m -> coefficient of q[ty][s] in output position m+1
    d_ik = nc.dram_tensor("ik", [96, WCOLS], bf16, kind="ExternalInput")
    d_wcv = nc.dram_tensor("wcv", [4, 128, 4, 3, H], bf16, kind="ExternalInput")
    d_wout = nc.dram_tensor("wout", [128, 4, 10], bf16, kind="ExternalInput")
    d_bias = nc.dram_tensor("biases", [128, 8, 4], f32, kind="ExternalInput")
    d_brep = nc.dram_tensor("brep", [128, NCHUNK, 10], f32, kind="ExternalInput")
    d_mask = nc.dram_tensor("mask", [128, 4], f32, kind="ExternalInput")
    d_z2n = nc.dram_tensor("z2n", [128, NCHUNK, 2], f32, kind="ExternalOutput")
    d_ld = nc.dram_tensor("ld", [NCHUNK, 4], f32, kind="ExternalOutput")

    with tile.TileContext(nc) as tc:
        with (
            tc.tile_pool(name="w", bufs=1) as wp,
            tc.tile_pool(name="act", bufs=1) as ap_,
            tc.tile_pool(name="ew", bufs=1) as ewp,
            tc.tile_pool(name="ps", bufs=8, space="PSUM") as pp,
        ):
            def mk(pool, shape, dtype, tag):
                return pool.tile(shape, dtype, tag=tag, name=tag)

            # ---- SBUF loads ----
            # issue DMAs from several engine queues in parallel (each
            # DMA_DIRECT2D costs ~0.6-1.3us of serial issue time per queue)
            z1g_sb = mk(ap_, [6, WCOLS], bf16, "z1g")
            nc.gpsimd.dma_start(z1g_sb[:], d_z1g.ap())
            wh6_sb = mk(wp, [6, H], bf16, "wh6")
            nc.gpsimd.dma_start(wh6_sb[:], d_wh6.ap())
            condT_sb = mk(ap_, [2 * COND, NB], bf16, "condT")
            nc.gpsimd.dma_start(condT_sb[:], d_condT.ap())
            wc1_sb = mk(wp, [2 * COND, H], bf16, "wc1")
            nc.gpsimd.dma_start(wc1_sb[:], d_wc1.ap())
            bias_sb = mk(wp, [128, 8, 4], f32, "bias")
            nc.gpsimd.dma_start(bias_sb[:], d_bias.ap())
            wc2_sb = mk(wp, [128, 4, H], bf16, "wc2")
            nc.sync.dma_start(wc2_sb[:], d_wc2.ap())
            vb_sb = mk(wp, [1, 3, H], bf16, "vb")
            nc.sync.dma_start(vb_sb[:], d_vb.ap())
            ik_sb = mk(wp, [96, WCOLS], bf16, "ik")
            nc.sync.dma_start(ik_sb[:], d_ik.ap())
            qw_sb = mk(wp, [128, 4, 3, H], bf16, "qw")
            for kt in range(4):
                nc.sync.dma_start(qw_sb[:, kt], d_qw.ap()[:, kt])
            wcv_sb = [mk(wp, [128, 4, 3, H], bf16, f"wcv_{l}") for l in range(4)]
            for l in range(4):
                nc.sync.dma_start(wcv_sb[l][:, 0:2], d_wcv.ap()[l, :, 0:2])
                nc.sync.dma_start(wcv_sb[l][:, 2:4], d_wcv.ap()[l, :, 2:4])
            wout_sb = mk(wp, [128, 4, 10], bf16, "wout")
            nc.sync.dma_start(wout_sb[:], d_wout.ap())
            brep_sb = mk(wp, [128, NCHUNK, 10], f32, "brep")
            nc.sync.dma_start(brep_sb[:], d_brep.ap())
            mask_sb = mk(wp, [128, 4], f32, "mask")
            nc.sync.dma_start(mask_sb[:], d_mask.ap())
            z2r_sb = mk(ewp, [128, NCHUNK, 2], f32, "z2r")
            nc.sync.dma_start(z2r_sb[:], d_z2r.ap())

            def bias_ap(idx, mt):
                return bias_sb[:, idx, mt : mt + 1]

            # ---- PE warm-up scratch (memset first so the PE can spin ASAP) ----
            wu_l = mk(ap_, [128, 128], bf16, "wu_l")
            wu_r = mk(ap_, [128, 512], bf16, "wu_r")
            nc.vector.memset(wu_l[:], 0.0)
            nc.vector.memset(wu_r[:], 0.0)

            # ---- activation ping-pong buffers (gap layout) ----
            # Only the gap guard columns need zeroing; valid columns are
            # always written before they are read.
            actB = [mk(ap_, [128, WCOLS], bf16, f"B{i}") for i in range(4)]
            actC = [mk(ap_, [128, WCOLS], bf16, f"C{i}") for i in range(4)]
            for ti, t_ in enumerate(actB + actC):
                v = t_[:].rearrange("p (s c) -> p s c", c=ST)
                eng = nc.vector if ti % 2 else nc.gpsimd
                eng.memset(v[:, :, 0:1], 0.0)
                eng.memset(v[:, :, ST - 1 : ST], 0.0)
            ones_sb = mk(ap_, [1, NB], bf16, "ones")
            nc.vector.memset(ones_sb[:], 1.0)

            def valid(tl, g=None):
                v = tl[:].rearrange("p (s c) -> p s c", c=ST)
                if g is None:
                    return v[:, :, 1 : 1 + T2]
                return v[:, g * GS : (g + 1) * GS, 1 : 1 + T2]

            # ---- PE warm-up spin ----
            # The PE HAM clock gate starts at 1.2 GHz and only releases to
            # 2.4 GHz after ~3.4us of sustained activity. Matmul on scratch
            # zeros while the weight DMAs stream in, so the real conv stack
            # runs warm from its first instruction.
            ps_w = mk(pp, [128, 512], f32, "ps")
            for i in range(16):
                nc.tensor.matmul(
                    ps_w[:], wu_l[:], wu_r[:], start=(i == 0), stop=(i == 15)
                )

            # preload the ACT transcendental tables so the elementwise tail
            # doesn't pay the table-swap latency
            scr = mk(ewp, [1, 4], f32, "scr")
            nc.scalar.activation(scr[:, 0:1], wu_l[:1, 0:1], AF.Exp)
            nc.scalar.activation(scr[:, 1:2], wu_l[:1, 0:1], AF.Tanh)
            nc.scalar.activation(scr[:, 2:3], wu_l[:1, 0:1], AF.Ln, bias=1.0)

            # ---- cond MLP: c2 = relu(W2 relu(W1 c + b1) + b2) ----
            c1_sb = [mk(ap_, [128, NB], bf16, f"c1_{i}") for i in range(4)]
            c2_sb = [mk(ap_, [128, NB], bf16, f"c2_{i}") for i in range(4)]
            for mt in range(4):
                ps = mk(pp, [128, 512], f32, "ps")
                nc.tensor.matmul(
                    ps[:, :NB],
                    wc1_sb[:, mt * 128 : (mt + 1) * 128],
                    condT_sb[:],
                    start=True,
                    stop=True,
                )
                nc.scalar.activation(
                    c1_sb[mt][:], ps[:, :NB], AF.Relu, bias=bias_ap(1, mt)
                )
            for mt in range(4):
                ps = mk(pp, [128, 512], f32, "ps")
                for kt in range(4):
                    nc.tensor.matmul(
                        ps[:, :NB],
                        wc2_sb[:, kt, mt * 128 : (mt + 1) * 128],
                        c1_sb[kt][:],
                        start=(kt == 0),
                        stop=(kt == 3),
                    )
                nc.scalar.activation(
                    c2_sb[mt][:], ps[:, :NB], AF.Relu, bias=bias_ap(2, mt)
                )

            WINDOWS = [(0, 15), (15, 15), (30, 2)]  # (sample base, n samples)

            def relu_copy(L, mt, wi, pv_, dsts, sb, ns):
                dv_ = dsts[mt][:].rearrange("p (s c) -> p s c", c=ST)[
                    :, sb : sb + ns, 1 : 1 + T2
                ]
                # split the PSUM->SBUF relu copies across ACT and DVE
                if (mt * 3 + wi) % 2 == 0:
                    nc.scalar.activation(dv_, pv_, AF.Relu, bias=bias_ap(3 + L, mt))
                else:
                    nc.vector.tensor_scalar(
                        dv_, pv_, bias_ap(3 + L, mt), 0.0, Alu.add, Alu.max
                    )

            # ---- conv0 ----
            # qT[ty*32+s, o]: per-sample cond/bias vectors, computed
            # transposed so they can be applied to the conv0 windows by a
            # single K=96 indicator matmul inside the PSUM accumulation:
            #   ty=0: (sum_k W0k_c) @ c2[s] + sum_k(W0k_h @ b_embed)
            #   ty=1: W00_c @ c2[s] + v0   (subtracted at t=0)
            #   ty=2: W02_c @ c2[s] + v2   (subtracted at t=31)
            psq = mk(pp, [96, 512], f32, "ps")
            for ty in range(3):
                for kt in range(4):
                    nc.tensor.matmul(
                        psq[ty * NB : (ty + 1) * NB, :],
                        c2_sb[kt][:],
                        qw_sb[:, kt, ty, :],
                        start=(kt == 0),
                        stop=False,
                    )
                nc.tensor.matmul(
                    psq[ty * NB : (ty + 1) * NB, :],
                    ones_sb[:],
                    vb_sb[:, ty, :],
                    start=False,
                    stop=True,
                )
            qT_sb = mk(ap_, [96, H], bf16, "qT")
            nc.vector.tensor_copy(qT_sb[:], psq[:])

            # h-half: single K=6 matmul per window (the 3 conv shifts are
            # baked into the replicated z1g rows); q applied via indicator
            for mt in range(4):
                pss = [mk(pp, [128, 512], f32, "ps") for _ in WINDOWS]
                for wi, (sb, ns) in enumerate(WINDOWS):
                    n = ns * ST - 2
                    base = sb * ST
                    nc.tensor.matmul(
                        pss[wi][:, :n],
                        wh6_sb[:, mt * 128 : (mt + 1) * 128],
                        z1g_sb[:, base : base + n],
                        start=True,
                        stop=False,
                    )
                    nc.tensor.matmul(
                        pss[wi][:, :n],
                        qT_sb[:, mt * 128 : (mt + 1) * 128],
                        ik_sb[:, base : base + n],
                        start=False,
                        stop=True,
                    )
                for wi, (sb, ns) in enumerate(WINDOWS):
                    pv3 = pss[wi][:, : ns * ST].rearrange("p (s c) -> p s c", c=ST)
                    relu_copy(0, mt, wi, pv3[:, :, 0:T2], actB, sb, ns)

            # ---- conv1..4 ----
            # Matmul moving operands must be single-free-dim, so each conv
            # matmul streams a contiguous window of the gap layout; outputs
            # at gap positions are garbage and simply never read back.
            for L in range(1, 5):
                srcs = [actB, actC, actB, actC][L - 1]
                wts = wcv_sb[L - 1][:]
                dsts = actB if L % 2 == 0 else actC
                for mt in range(4):
                    pss = [mk(pp, [128, 512], f32, "ps") for _ in WINDOWS]
                    nacc = 12
                    i = 0
                    # kt outer: matches weight-DMA arrival order
                    for kt in range(4):
                        for k in range(3):
                            lhsT = wts[:, kt, k, mt * 128 : (mt + 1) * 128]
                            for wi, (sb, ns) in enumerate(WINDOWS):
                                n = ns * ST - 2
                                base = sb * ST + k
                                nc.tensor.matmul(
                                    pss[wi][:, :n],
                                    lhsT,
                                    srcs[kt][:, base : base + n],
                                    start=(i == 0),
                                    stop=(i == nacc - 1),
                                )
                            i += 1
                    for wi, (sb, ns) in enumerate(WINDOWS):
                        pv3 = pss[wi][:, : ns * ST].rearrange(
                            "p (s c) -> p s c", c=ST
                        )
                        relu_copy(L, mt, wi, pv3[:, :, 0:T2], dsts, sb, ns)
            srcs = actB

            # ---- compact the final activation (drop gap columns) ----
            h5 = [mk(ap_, [128, NB * T2], bf16, f"h5_{i}") for i in range(4)]
            for kt in range(4):
                eng = nc.scalar if kt % 2 == 0 else nc.vector
                if kt % 2 == 0:
                    nc.scalar.copy(
                        h5[kt][:].rearrange("p (s t) -> p s t", t=T2),
                        valid(srcs[kt]),
                    )
                else:
                    nc.vector.tensor_copy(
                        h5[kt][:].rearrange("p (s t) -> p s t", t=T2),
                        valid(srcs[kt]),
                    )

            # ---- final projection (transposed): out[col, 10] ----
            pso = mk(pp, [128, 512], f32, "ps")
            for j in range(NCHUNK):
                for kt in range(4):
                    nc.tensor.matmul(
                        pso[:, j * 10 : (j + 1) * 10],
                        h5[kt][:, j * 128 : (j + 1) * 128],
                        wout_sb[:, kt],
                        start=(kt == 0),
                        stop=(kt == 3),
                    )

            # ---- NLSQ elementwise tail ----
            def ew(tag):
                return mk(ewp, [128, NCHUNK, 2], f32, tag)

            params = mk(ewp, [128, NCHUNK, 10], f32, "params")
            nc.vector.tensor_add(
                params[:],
                pso[:, : NCHUNK * 10].rearrange("p (j q) -> p j q", q=10),
                brep_sb[:],
            )
            pv = params[:].rearrange("p j (a q) -> p j a q", q=5)
            P0, P1, P2, P3, P4 = (pv[:, :, :, i] for i in range(5))

            loga_sb = mk(ewp, [128, 1], f32, "loga")
            nc.vector.memset(loga_sb[:], LOG_A)

            # group ACT functions (Exp x3, then Tanh, Ln last) to minimize
            # activation-table reloads
            u = ew("u")
            nc.vector.tensor_tensor(u[:], P1, P3, op=Alu.subtract)
            b_ = ew("b_")
            nc.scalar.activation(b_[:], P1, AF.Exp, scale=0.4)
            d_ = ew("d_")
            nc.scalar.activation(d_[:], P3, AF.Exp, scale=0.4)
            e = ew("e")
            nc.scalar.activation(e[:], u[:], AF.Exp, scale=0.4, bias=loga_sb[:])
            th = ew("th")
            nc.scalar.activation(th[:], P2, AF.Tanh, scale=0.3)
            c_ = ew("c_")
            nc.vector.tensor_mul(c_[:], th[:], e[:])
            t1 = ew("t1")
            nc.vector.tensor_mul(t1[:], d_[:], z2r_sb[:])
            arg = ew("arg")
            nc.vector.tensor_add(arg[:], t1[:], P4)
            sq = ew("sq")
            nc.vector.tensor_mul(sq[:], arg[:], arg[:])
            den = ew("den")
            nc.vector.tensor_scalar_add(den[:], sq[:], 1.0)
            rcp = ew("rcp")
            nc.vector.reciprocal(rcp[:], den[:])
            t2 = ew("t2")
            nc.vector.tensor_mul(t2[:], b_[:], z2r_sb[:])
            t3 = ew("t3")
            nc.vector.tensor_mul(t3[:], c_[:], rcp[:])
            s1 = ew("s1")
            nc.vector.tensor_add(s1[:], P0, t2[:])
            z2n_sb = ew("z2n_sb")
            nc.vector.tensor_add(z2n_sb[:], s1[:], t3[:])
            nc.sync.dma_start(d_z2n.ap(), z2n_sb[:])

            t4 = ew("t4")
            nc.vector.tensor_mul(t4[:], c_[:], d_[:])
            t5 = ew("t5")
            nc.vector.tensor_mul(t5[:], t4[:], arg[:])
            t6 = ew("t6")
            nc.vector.tensor_mul(t6[:], t5[:], rcp[:])
            t7 = ew("t7")
            nc.vector.tensor_mul(t7[:], t6[:], rcp[:])
            inner = ew("inner")
            nc.vector.scalar_tensor_tensor(
                inner[:], t7[:], -2.0, b_[:], op0=Alu.mult, op1=Alu.add
            )
            lg = ew("lg")
            nc.scalar.activation(lg[:], inner[:], AF.Ln)

            lg2 = mk(ewp, [128, NCHUNK], f32, "lg2")
            nc.vector.tensor_add(lg2[:], lg[:, :, 0], lg[:, :, 1])
            psl = mk(pp, [128, 512], f32, "ps")
            nc.tensor.matmul(
                psl[:NCHUNK, :4], lg2[:], mask_sb[:], start=True, stop=True
            )
            ld_sb = mk(ewp, [NCHUNK, 4], f32, "ld_sb")
            nc.vector.tensor_copy(ld_sb[:], psl[:NCHUNK, :4])
            nc.sync.dma_start(d_ld.ap(), ld_sb[:])

    nc.compile()
    return nc


def _get_program():
    if "nc" not in _CACHE:
        _CACHE["nc"] = _build_program()
    return _CACHE["nc"]


def _host_inputs(inputs):
    import ml_dtypes

    bf16 = ml_dtypes.bfloat16
    f32 = np.float32

    x = np.asarray(inputs["x"], f32)
    cond = np.asarray(inputs["cond"], f32)

    wc1T = np.ascontiguousarray(inputs["w_c1"].T).astype(bf16)
    # partition-major packing: [128, ...] with large contiguous per-partition
    # chunks so SBUF DMAs are few big descriptors per partition
    wc2T = np.ascontiguousarray(
        inputs["w_c2"].T.reshape(4, 128, H).transpose(1, 0, 2)
    ).astype(bf16)
    # conv0 h-half collapsed through the rank-2 embedding; rows (k*2+d)
    w0 = np.asarray(inputs["w_conv0"], f32)
    we2 = np.asarray(inputs["w_embed"], f32)[:, :2]
    wh6 = np.ascontiguousarray(
        np.einsum("ock,cd->kdo", w0[:, :H, :], we2).reshape(6, H)
    ).astype(bf16)
    # conv0 cond-half collapsed to per-sample vectors (types: sum_k, k0, k2)
    S = w0[:, H:, :]
    mats = np.stack([(S[:, :, 0] + S[:, :, 1] + S[:, :, 2]).T, S[:, :, 0].T, S[:, :, 2].T])
    qw = np.ascontiguousarray(
        mats.reshape(3, 4, 128, H).transpose(2, 1, 0, 3)
    ).astype(bf16)
    # b_embed folded through conv0 (rank-1 terms per type)
    be = np.asarray(inputs["b_embed"], f32)
    v = np.stack([w0[:, :H, k] @ be for k in range(3)])  # [3, H(out)]
    vb = np.ascontiguousarray(
        np.stack([v[0] + v[1] + v[2], v[0], v[2]])[None, :, :]
    ).astype(bf16)
    wcv = np.stack(
        [
            np.ascontiguousarray(
                np.transpose(inputs[f"w_conv{i}"], (1, 2, 0))
                .reshape(4, 128, 3, H)
                .transpose(1, 0, 2, 3)
            )
            for i in (1, 2, 3, 4)
        ]
    ).astype(bf16)
    woutT = np.ascontiguousarray(
        inputs["w_out"].T.reshape(4, 128, 10).transpose(1, 0, 2)
    ).astype(bf16)
    bias_all = np.stack(
        [inputs["b_embed"], inputs["b_c1"], inputs["b_c2"]]
        + [inputs[f"b_conv{i}"] for i in range(5)]
    ).astype(f32)
    bias_pack = np.ascontiguousarray(
        bias_all.reshape(8, 4, 128).transpose(2, 0, 1)
    ).astype(f32)
    brep = np.ascontiguousarray(
        np.broadcast_to(inputs["b_out"].astype(f32), (128, NCHUNK, 10))
    )
    mask = np.zeros((128, 4), f32)
    mask[np.arange(128), np.arange(128) // 32] = 1.0
    ik = np.zeros((96, WCOLS), f32)
    for s in range(NB):
        ik[s, ST * s : ST * s + T2] = 1.0
        ik[NB + s, ST * s] = -1.0
        ik[2 * NB + s, ST * s + T2 - 1] = -1.0
    ik = ik.astype(bf16)

    in_maps = []
    for c in range(NCORES):
        xs = x[c * NB : (c + 1) * NB]
        z1 = xs[:, :T2]
        z2 = xs[:, T2:]
        z1p = np.zeros((2, WCOLS + 2), np.float32)
        z1p[:, :WCOLS].reshape(2, NB, ST)[:, :, 1 : 1 + T2] = z1.transpose(2, 0, 1)
        z1g = np.stack(
            [z1p[dd, k : k + WCOLS] for k in range(3) for dd in range(2)]
        ).astype(bf16)
        z2r = np.ascontiguousarray(
            z2.reshape(NCHUNK, 4, T2, 2).transpose(1, 2, 0, 3)
        ).reshape(128, NCHUNK, 2)
        condT = np.ascontiguousarray(
            cond[c * NB : (c + 1) * NB].reshape(NB, 2 * COND).T
        ).astype(bf16)
        in_maps.append(
            dict(
                z1g=z1g,
                z2r=z2r,
                condT=condT,
                wc1=wc1T,
                wc2=wc2T,
                wh6=wh6,
                qw=qw,
                vb=vb,
                ik=ik,
                wcv=wcv,
                wout=woutT,
                biases=bias_pack,
                brep=brep,
                mask=mask,
            )
        )
    return in_maps


def _assemble_output(x, results):
    z = np.empty((B, T, D), np.float32)
    ld = np.empty((B,), np.float32)
    for c in range(NCORES):
        z[c * NB : (c + 1) * NB, :T2] = x[c * NB : (c + 1) * NB, :T2]
        z2n = np.asarray(results[c]["z2n"], np.float32)
        z[c * NB : (c + 1) * NB, T2:] = (
            z2n.reshape(4, T2, NCHUNK, 2).transpose(2, 0, 1, 3).reshape(NB, T2, 2)
        )
        ld[c * NB : (c + 1) * NB] = np.asarray(results[c]["ld"], np.float32).reshape(
            NB
        )
    return z, ld


def run(inputs, trace=False, trace_cores=None):
    """Run on 8 NeuronCores; returns ((z, logdet), BassKernelResults)."""
    from concourse.bass_utils import run_bass_kernel_spmd

    nc = _get_program()
    in_maps = _host_inputs(inputs)
    res = run_bass_kernel_spmd(
        nc,
        in_maps,
        list(range(NCORES)),
        trace=trace,
        trace_cores=trace_cores if trace_cores is not None else list(range(NCORES)),
    )
    x = np.asarray(inputs["x"], np.float32)
    return _assemble_output(x, res.results), res


def kernel(**inputs):
    (z, ld), _ = run(inputs, trace=False)
    return z, ld


if __name__ == "__main__":
    print("build only:", _get_program())


# revision 47
# speedup vs baseline: 1.1025x; 1.0342x over previous
"""Trainium2 Bass kernel for nn_NlsqCond (ConvFlow NLSQ coupling layer).

Strategy: pure data parallel over batch B=256 -> 32 samples per core on 8
NeuronCores. Convs are computed as 3 shifted matmuls accumulating in PSUM
over a gap-column activation layout (34 columns per sample, zero guard
columns), so the k=3/pad=1 conv needs no boundary special-casing. Weights
are transposed host-side into lhsT layout and cast to bf16 (fp32 PSUM
accumulation); measured end-to-end error vs fp32 reference is ~1e-4.
The final projection is computed transposed ([cols, 10]) so the NLSQ
elementwise tail runs with full 128-partition parallelism; the per-sample
logdet partition-reduction is done with a small mask matmul.
"""

import math

import numpy as np

B, T, D, H, COND = 256, 64, 2, 512, 8
T2 = T // 2                      # 32
NCORES = 8
NB = B // NCORES                 # 32 samples per core
ST = T2 + 2                      # 34: per-sample column stride (zero gaps)
WCOLS = NB * ST                  # 1088
NG = 2                           # PSUM column groups per matmul set
GS = NB // NG                    # 16 samples per group
NCHUNK = NB * T2 // 128          # 8 column chunks of 128 for final proj
LOG_A = math.log(8.0 * math.sqrt(3.0) / 9.0 - 0.05)

_CACHE = {}


def _build_program():
    import concourse.bacc as bacc
    import concourse.mybir as mybir
    import concourse.tile as tile

    f32 = mybir.dt.float32
    bf16 = mybir.dt.bfloat16
    AF = mybir.ActivationFunctionType
    Alu = mybir.AluOpType

    nc = bacc.Bacc("TRN2", target_bir_lowering=False, debug=False)

    # ---- DRAM I/O ----
    # z1 in gap layout (zero guard columns), rows replicated for the 3 conv
    # shifts: row (k*2+d) col m = z1_gap[d, m+k]. Feeds conv0's h-half as a
    # single K=6 matmul per window.
    d_z1g = nc.dram_tensor("z1g", [128, WCOLS], bf16, kind="ExternalInput")
    d_z2r = nc.dram_tensor("z2r", [128, NCHUNK, 2], f32, kind="ExternalInput")
    d_condT = nc.dram_tensor("condT", [128, NB], bf16, kind="ExternalInput")
    d_wc1 = nc.dram_tensor("wc1", [128, H], bf16, kind="ExternalInput")
    # conv/linear weights packed partition-major so each DMA moves one large
    # contiguous chunk per partition (descriptor-rate, not bandwidth, limits
    # small-row DMAs)
    d_wc2 = nc.dram_tensor("wc2", [128, 4, H], bf16, kind="ExternalInput")
    # conv0 h-half collapsed to rank 6: wh6[k*2+d] = (w_conv0[:, :H, k] @ we).T
    d_wh6 = nc.dram_tensor("wh6", [128, H], bf16, kind="ExternalInput")
    # conv0 cond-half collapsed to per-sample vectors: types (sum_k, k=0, k=2)
    d_qw = nc.dram_tensor("qw", [128, 4, 3, H], bf16, kind="ExternalInput")
    # b_embed fold rank-1 terms per type
    d_vb = nc.dram_tensor("vb", [1, 3, H], bf16, kind="ExternalInput")
    # signed indicator matrix applying q to the conv0 windows on the PE:
    # row ty*32+s, col m -> coefficient of q[ty][s] in output position m+1
    d_ik = nc.dram_tensor("ik", [128, WCOLS], bf16, kind="ExternalInput")
    d_wcv = nc.dram_tensor("wcv", [4, 128, 4, 3, H], bf16, kind="ExternalInput")
    d_wout = nc.dram_tensor("wout", [128, 4, 10], bf16, kind="ExternalInput")
    d_bias = nc.dram_tensor("biases", [128, 8, 4], f32, kind="ExternalInput")
    d_brep = nc.dram_tensor("brep", [128, NCHUNK, 10], f32, kind="ExternalInput")
    d_mask = nc.dram_tensor("mask", [128, 4], f32, kind="ExternalInput")
    d_z2n = nc.dram_tensor("z2n", [128, NCHUNK, 2], f32, kind="ExternalOutput")
    d_ld = nc.dram_tensor("ld", [NCHUNK, 4], f32, kind="ExternalOutput")

    with tile.TileContext(nc) as tc:
        with (
            tc.tile_pool(name="w", bufs=1) as wp,
            tc.tile_pool(name="act", bufs=1) as ap_,
            tc.tile_pool(name="ew", bufs=1) as ewp,
            tc.tile_pool(name="ps", bufs=8, space="PSUM") as pp,
        ):
            def mk(pool, shape, dtype, tag):
                return pool.tile(shape, dtype, tag=tag, name=tag)

            # ---- SBUF loads ----
            # issue DMAs from several engine queues in parallel (each
            # DMA_DIRECT2D costs ~0.6-1.3us of serial issue time per queue)
            z1g_sb = mk(ap_, [128, WCOLS], bf16, "z1g")
            nc.gpsimd.dma_start(z1g_sb[:], d_z1g.ap())
            wh6_sb = mk(wp, [128, H], bf16, "wh6")
            nc.gpsimd.dma_start(wh6_sb[:], d_wh6.ap())
            condT_sb = mk(ap_, [128, NB], bf16, "condT")
            nc.gpsimd.dma_start(condT_sb[:], d_condT.ap())
            wc1_sb = mk(wp, [128, H], bf16, "wc1")
            nc.gpsimd.dma_start(wc1_sb[:], d_wc1.ap())
            bias_sb = mk(wp, [128, 8, 4], f32, "bias")
            nc.gpsimd.dma_start(bias_sb[:], d_bias.ap())
            wc2_sb = mk(wp, [128, 4, H], bf16, "wc2")
            nc.sync.dma_start(wc2_sb[:], d_wc2.ap())
            vb_sb = mk(wp, [1, 3, H], bf16, "vb")
            nc.sync.dma_start(vb_sb[:], d_vb.ap())
            ik_sb = mk(wp, [128, WCOLS], bf16, "ik")
            nc.sync.dma_start(ik_sb[:], d_ik.ap())
            qw_sb = mk(wp, [128, 4, 3, H], bf16, "qw")
            for kt in range(4):
                nc.sync.dma_start(qw_sb[:, kt], d_qw.ap()[:, kt])
            wcv_sb = [mk(wp, [128, 4, 3, H], bf16, f"wcv_{l}") for l in range(4)]
            for l in range(4):
                nc.sync.dma_start(wcv_sb[l][:, 0:2], d_wcv.ap()[l, :, 0:2])
                nc.sync.dma_start(wcv_sb[l][:, 2:4], d_wcv.ap()[l, :, 2:4])
            wout_sb = mk(wp, [128, 4, 10], bf16, "wout")
            nc.sync.dma_start(wout_sb[:], d_wout.ap())
            brep_sb = mk(wp, [128, NCHUNK, 10], f32, "brep")
            nc.sync.dma_start(brep_sb[:], d_brep.ap())
            mask_sb = mk(wp, [128, 4], f32, "mask")
            nc.sync.dma_start(mask_sb[:], d_mask.ap())
            z2r_sb = mk(ewp, [128, NCHUNK, 2], f32, "z2r")
            nc.sync.dma_start(z2r_sb[:], d_z2r.ap())

            def bias_ap(idx, mt):
                return bias_sb[:, idx, mt : mt + 1]

            # ---- PE warm-up scratch (memset first so the PE can spin ASAP) ----
            wu_l = mk(ap_, [128, 128], bf16, "wu_l")
            wu_r = mk(ap_, [128, 512], bf16, "wu_r")
            nc.vector.memset(wu_l[:], 0.0)
            nc.vector.memset(wu_r[:], 0.0)

            # ---- activation ping-pong buffers (gap layout) ----
            # Only the gap guard columns need zeroing; valid columns are
            # always written before they are read.
            actB = [mk(ap_, [128, WCOLS], bf16, f"B{i}") for i in range(4)]
            actC = [mk(ap_, [128, WCOLS], bf16, f"C{i}") for i in range(4)]
            for ti, t_ in enumerate(actB + actC):
                v = t_[:].rearrange("p (s c) -> p s c", c=ST)
                eng = nc.vector if ti % 2 else nc.gpsimd
                eng.memset(v[:, :, 0:1], 0.0)
                eng.memset(v[:, :, ST - 1 : ST], 0.0)
            ones_sb = mk(ap_, [1, NB], bf16, "ones")
            nc.vector.memset(ones_sb[:], 1.0)

            def valid(tl, g=None):
                v = tl[:].rearrange("p (s c) -> p s c", c=ST)
                if g is None:
                    return v[:, :, 1 : 1 + T2]
                return v[:, g * GS : (g + 1) * GS, 1 : 1 + T2]

            # ---- PE warm-up spin ----
            # The PE HAM clock gate starts at 1.2 GHz and only releases to
            # 2.4 GHz after ~3.4us of sustained activity. Matmul on scratch
            # zeros while the weight DMAs stream in, so the real conv stack
            # runs warm from its first instruction.
            ps_w = mk(pp, [128, 512], f32, "ps")
            for i in range(16):
                nc.tensor.matmul(
                    ps_w[:], wu_l[:], wu_r[:], start=(i == 0), stop=(i == 15)
                )

            # preload the ACT transcendental tables so the elementwise tail
            # doesn't pay the table-swap latency
            scr = mk(ewp, [1, 4], f32, "scr")
            nc.scalar.activation(scr[:, 0:1], wu_l[:1, 0:1], AF.Exp)
            nc.scalar.activation(scr[:, 1:2], wu_l[:1, 0:1], AF.Tanh)
            nc.scalar.activation(scr[:, 2:3], wu_l[:1, 0:1], AF.Ln, bias=1.0)

            # ---- cond MLP: c2 = relu(W2 relu(W1 c + b1) + b2) ----
            c1_sb = [mk(ap_, [128, NB], bf16, f"c1_{i}") for i in range(4)]
            c2_sb = [mk(ap_, [128, NB], bf16, f"c2_{i}") for i in range(4)]
            for mt in range(4):
                ps = mk(pp, [128, 512], f32, "ps")
                nc.tensor.matmul(
                    ps[:, :NB],
                    wc1_sb[:, mt * 128 : (mt + 1) * 128],
                    condT_sb[:],
                    start=True,
                    stop=True,
                )
                nc.scalar.activation(
                    c1_sb[mt][:], ps[:, :NB], AF.Relu, bias=bias_ap(1, mt)
                )
            for mt in range(4):
                ps = mk(pp, [128, 512], f32, "ps")
                for kt in range(4):
                    nc.tensor.matmul(
                        ps[:, :NB],
                        wc2_sb[:, kt, mt * 128 : (mt + 1) * 128],
                        c1_sb[kt][:],
                        start=(kt == 0),
                        stop=(kt == 3),
                    )
                nc.scalar.activation(
                    c2_sb[mt][:], ps[:, :NB], AF.Relu, bias=bias_ap(2, mt)
                )

            WINDOWS = [(0, 15), (15, 15), (30, 2)]  # (sample base, n samples)

            def relu_copy(L, mt, wi, pv_, dsts, sb, ns):
                dv_ = dsts[mt][:].rearrange("p (s c) -> p s c", c=ST)[
                    :, sb : sb + ns, 1 : 1 + T2
                ]
                # split the PSUM->SBUF relu copies across ACT and DVE
                if (mt * 3 + wi) % 2 == 0:
                    nc.scalar.activation(dv_, pv_, AF.Relu, bias=bias_ap(3 + L, mt))
                else:
                    nc.vector.tensor_scalar(
                        dv_, pv_, bias_ap(3 + L, mt), 0.0, Alu.add, Alu.max
                    )

            # ---- conv0 ----
            # qT[ty*32+s, o]: per-sample cond/bias vectors, computed
            # transposed so they can be applied to the conv0 windows by a
            # single K=96 indicator matmul inside the PSUM accumulation:
            #   ty=0: (sum_k W0k_c) @ c2[s] + sum_k(W0k_h @ b_embed)
            #   ty=1: W00_c @ c2[s] + v0   (subtracted at t=0)
            #   ty=2: W02_c @ c2[s] + v2   (subtracted at t=31)
            psq = mk(pp, [96, 512], f32, "ps")
            for ty in range(3):
                for kt in range(4):
                    nc.tensor.matmul(
                        psq[ty * NB : (ty + 1) * NB, :],
                        c2_sb[kt][:],
                        qw_sb[:, kt, ty, :],
                        start=(kt == 0),
                        stop=False,
                    )
                nc.tensor.matmul(
                    psq[ty * NB : (ty + 1) * NB, :],
                    ones_sb[:],
                    vb_sb[:, ty, :],
                    start=False,
                    stop=True,
                )
            qT_sb = mk(ap_, [128, H], bf16, "qT")
            nc.vector.memset(qT_sb[96:128, :], 0.0)
            nc.vector.tensor_copy(qT_sb[:96, :], psq[:])

            # h-half: single K=6 matmul per window (the 3 conv shifts are
            # baked into the replicated z1g rows); q applied via indicator
            for mt in range(4):
                pss = [mk(pp, [128, 512], f32, "ps") for _ in WINDOWS]
                for wi, (sb, ns) in enumerate(WINDOWS):
                    n = ns * ST - 2
                    base = sb * ST
                    nc.tensor.matmul(
                        pss[wi][:, :n],
                        wh6_sb[:, mt * 128 : (mt + 1) * 128],
                        z1g_sb[:, base : base + n],
                        start=True,
                        stop=False,
                    )
                    nc.tensor.matmul(
                        pss[wi][:, :n],
                        qT_sb[:, mt * 128 : (mt + 1) * 128],
                        ik_sb[:, base : base + n],
                        start=False,
                        stop=True,
                    )
                for wi, (sb, ns) in enumerate(WINDOWS):
                    pv3 = pss[wi][:, : ns * ST].rearrange("p (s c) -> p s c", c=ST)
                    relu_copy(0, mt, wi, pv3[:, :, 0:T2], actB, sb, ns)

            # ---- conv1..4 ----
            # Matmul moving operands must be single-free-dim, so each conv
            # matmul streams a contiguous window of the gap layout; outputs
            # at gap positions are garbage and simply never read back.
            for L in range(1, 5):
                srcs = [actB, actC, actB, actC][L - 1]
                wts = wcv_sb[L - 1][:]
                dsts = actB if L % 2 == 0 else actC
                for mt in range(4):
                    pss = [mk(pp, [128, 512], f32, "ps") for _ in WINDOWS]
                    nacc = 12
                    i = 0
                    # kt outer: matches weight-DMA arrival order
                    for kt in range(4):
                        for k in range(3):
                            lhsT = wts[:, kt, k, mt * 128 : (mt + 1) * 128]
                            for wi, (sb, ns) in enumerate(WINDOWS):
                                n = ns * ST - 2
                                base = sb * ST + k
                                nc.tensor.matmul(
                                    pss[wi][:, :n],
                                    lhsT,
                                    srcs[kt][:, base : base + n],
                                    start=(i == 0),
                                    stop=(i == nacc - 1),
                                )
                            i += 1
                    for wi, (sb, ns) in enumerate(WINDOWS):
                        pv3 = pss[wi][:, : ns * ST].rearrange(
                            "p (s c) -> p s c", c=ST
                        )
                        relu_copy(L, mt, wi, pv3[:, :, 0:T2], dsts, sb, ns)
            srcs = actB

            # ---- compact the final activation (drop gap columns) ----
            h5 = [mk(ap_, [128, NB * T2], bf16, f"h5_{i}") for i in range(4)]
            for kt in range(4):
                eng = nc.scalar if kt % 2 == 0 else nc.vector
                if kt % 2 == 0:
                    nc.scalar.copy(
                        h5[kt][:].rearrange("p (s t) -> p s t", t=T2),
                        valid(srcs[kt]),
                    )
                else:
                    nc.vector.tensor_copy(
                        h5[kt][:].rearrange("p (s t) -> p s t", t=T2),
                        valid(srcs[kt]),
                    )

            # ---- final projection (transposed): out[col, 10] ----
            pso = mk(pp, [128, 512], f32, "ps")
            for j in range(NCHUNK):
                for kt in range(4):
                    nc.tensor.matmul(
                        pso[:, j * 10 : (j + 1) * 10],
                        h5[kt][:, j * 128 : (j + 1) * 128],
                        wout_sb[:, kt],
                        start=(kt == 0),
                        stop=(kt == 3),
                    )

            # ---- NLSQ elementwise tail ----
            def ew(tag):
                return mk(ewp, [128, NCHUNK, 2], f32, tag)

            params = mk(ewp, [128, NCHUNK, 10], f32, "params")
            nc.vector.tensor_add(
                params[:],
                pso[:, : NCHUNK * 10].rearrange("p (j q) -> p j q", q=10),
                brep_sb[:],
            )
            pv = params[:].rearrange("p j (a q) -> p j a q", q=5)
            P0, P1, P2, P3, P4 = (pv[:, :, :, i] for i in range(5))

            loga_sb = mk(ewp, [128, 1], f32, "loga")
            nc.vector.memset(loga_sb[:], LOG_A)

            # group ACT functions (Exp x3, then Tanh, Ln last) to minimize
            # activation-table reloads
            u = ew("u")
            nc.vector.tensor_tensor(u[:], P1, P3, op=Alu.subtract)
            b_ = ew("b_")
            nc.scalar.activation(b_[:], P1, AF.Exp, scale=0.4)
            d_ = ew("d_")
            nc.scalar.activation(d_[:], P3, AF.Exp, scale=0.4)
            e = ew("e")
            nc.scalar.activation(e[:], u[:], AF.Exp, scale=0.4, bias=loga_sb[:])
            th = ew("th")
            nc.scalar.activation(th[:], P2, AF.Tanh, scale=0.3)
            c_ = ew("c_")
            nc.vector.tensor_mul(c_[:], th[:], e[:])
            t1 = ew("t1")
            nc.vector.tensor_mul(t1[:], d_[:], z2r_sb[:])
            arg = ew("arg")
            nc.vector.tensor_add(arg[:], t1[:], P4)
            sq = ew("sq")
            nc.vector.tensor_mul(sq[:], arg[:], arg[:])
            den = ew("den")
            nc.vector.tensor_scalar_add(den[:], sq[:], 1.0)
            rcp = ew("rcp")
            nc.vector.reciprocal(rcp[:], den[:])
            t2 = ew("t2")
            nc.vector.tensor_mul(t2[:], b_[:], z2r_sb[:])
            t3 = ew("t3")
            nc.vector.tensor_mul(t3[:], c_[:], rcp[:])
            s1 = ew("s1")
            nc.vector.tensor_add(s1[:], P0, t2[:])
            z2n_sb = ew("z2n_sb")
            nc.vector.tensor_add(z2n_sb[:], s1[:], t3[:])
            nc.sync.dma_start(d_z2n.ap(), z2n_sb[:])

            t4 = ew("t4")
            nc.vector.tensor_mul(t4[:], c_[:], d_[:])
            t5 = ew("t5")
            nc.vector.tensor_mul(t5[:], t4[:], arg[:])
            t6 = ew("t6")
            nc.vector.tensor_mul(t6[:], t5[:], rcp[:])
            t7 = ew("t7")
            nc.vector.tensor_mul(t7[:], t6[:], rcp[:])
            inner = ew("inner")
            nc.vector.scalar_tensor_tensor(
                inner[:], t7[:], -2.0, b_[:], op0=Alu.mult, op1=Alu.add
            )
            lg = ew("lg")
            nc.scalar.activation(lg[:], inner[:], AF.Ln)

            lg2 = mk(ewp, [128, NCHUNK], f32, "lg2")
            nc.vector.tensor_add(lg2[:], lg[:, :, 0], lg[:, :, 1])
            psl = mk(pp, [128, 512], f32, "ps")
            nc.tensor.matmul(
                psl[:NCHUNK, :4], lg2[:], mask_sb[:], start=True, stop=True
            )
            ld_sb = mk(ewp, [NCHUNK, 4], f32, "ld_sb")
            nc.vector.tensor_copy(ld_sb[:], psl[:NCHUNK, :4])
            nc.sync.dma_start(d_ld.ap(), ld_sb[:])

    nc.compile()
    return nc


def _get_program():
    if "nc" not in _CACHE:
        _CACHE["nc"] = _build_program()
    return _CACHE["nc"]


def _host_inputs(inputs):
    import ml_dtypes

    bf16 = ml_dtypes.bfloat16
    f32 = np.float32

    x = np.asarray(inputs["x"], f32)
    cond = np.asarray(inputs["cond"], f32)

    wc1T = np.zeros((128, H), f32)
    wc1T[: 2 * COND] = inputs["w_c1"].T
    wc1T = wc1T.astype(bf16)
    # partition-major packing: [128, ...] with large contiguous per-partition
    # chunks so SBUF DMAs are few big descriptors per partition
    wc2T = np.ascontiguousarray(
        inputs["w_c2"].T.reshape(4, 128, H).transpose(1, 0, 2)
    ).astype(bf16)
    # conv0 h-half collapsed through the rank-2 embedding; rows (k*2+d)
    w0 = np.asarray(inputs["w_conv0"], f32)
    we2 = np.asarray(inputs["w_embed"], f32)[:, :2]
    wh6 = np.zeros((128, H), f32)
    wh6[:6] = np.einsum("ock,cd->kdo", w0[:, :H, :], we2).reshape(6, H)
    wh6 = wh6.astype(bf16)
    # conv0 cond-half collapsed to per-sample vectors (types: sum_k, k0, k2)
    S = w0[:, H:, :]
    mats = np.stack([(S[:, :, 0] + S[:, :, 1] + S[:, :, 2]).T, S[:, :, 0].T, S[:, :, 2].T])
    qw = np.ascontiguousarray(
        mats.reshape(3, 4, 128, H).transpose(2, 1, 0, 3)
    ).astype(bf16)
    # b_embed folded through conv0 (rank-1 terms per type)
    be = np.asarray(inputs["b_embed"], f32)
    v = np.stack([w0[:, :H, k] @ be for k in range(3)])  # [3, H(out)]
    vb = np.ascontiguousarray(
        np.stack([v[0] + v[1] + v[2], v[0], v[2]])[None, :, :]
    ).astype(bf16)
    wcv = np.stack(
        [
            np.ascontiguousarray(
                np.transpose(inputs[f"w_conv{i}"], (1, 2, 0))
                .reshape(4, 128, 3, H)
                .transpose(1, 0, 2, 3)
            )
            for i in (1, 2, 3, 4)
        ]
    ).astype(bf16)
    woutT = np.ascontiguousarray(
        inputs["w_out"].T.reshape(4, 128, 10).transpose(1, 0, 2)
    ).astype(bf16)
    bias_all = np.stack(
        [inputs["b_embed"], inputs["b_c1"], inputs["b_c2"]]
        + [inputs[f"b_conv{i}"] for i in range(5)]
    ).astype(f32)
    bias_pack = np.ascontiguousarray(
        bias_all.reshape(8, 4, 128).transpose(2, 0, 1)
    ).astype(f32)
    brep = np.ascontiguousarray(
        np.broadcast_to(inputs["b_out"].astype(f32), (128, NCHUNK, 10))
    )
    mask = np.zeros((128, 4), f32)
    mask[np.arange(128), np.arange(128) // 32] = 1.0
    ik = np.zeros((128, WCOLS), f32)
    for s in range(NB):
        ik[s, ST * s : ST * s + T2] = 1.0
        ik[NB + s, ST * s] = -1.0
        ik[2 * NB + s, ST * s + T2 - 1] = -1.0
    ik = ik.astype(bf16)

    in_maps = []
    for c in range(NCORES):
        xs = x[c * NB : (c + 1) * NB]
        z1 = xs[:, :T2]
        z2 = xs[:, T2:]
        z1p = np.zeros((2, WCOLS + 2), np.float32)
        z1p[:, :WCOLS].reshape(2, NB, ST)[:, :, 1 : 1 + T2] = z1.transpose(2, 0, 1)
        z1g = np.zeros((128, WCOLS), np.float32)
        z1g[:6] = np.stack(
            [z1p[dd, k : k + WCOLS] for k in range(3) for dd in range(2)]
        )
        z1g = z1g.astype(bf16)
        z2r = np.ascontiguousarray(
            z2.reshape(NCHUNK, 4, T2, 2).transpose(1, 2, 0, 3)
        ).reshape(128, NCHUNK, 2)
        condT = np.zeros((128, NB), np.float32)
        condT[: 2 * COND] = cond[c * NB : (c + 1) * NB].reshape(NB, 2 * COND).T
        condT = condT.astype(bf16)
        in_maps.append(
            dict(
                z1g=z1g,
                z2r=z2r,
                condT=condT,
                wc1=wc1T,
                wc2=wc2T,
                wh6=wh6,
                qw=qw,
                vb=vb,
                ik=ik,
                wcv=wcv,
                wout=woutT,
                biases=bias_pack,
                brep=brep,
                mask=mask,
            )
        )
    return in_maps


def _assemble_output(x, results):
    z = np.empty((B, T, D), np.float32)
    ld = np.empty((B,), np.float32)
    for c in range(NCORES):
        z[c * NB : (c + 1) * NB, :T2] = x[c * NB : (c + 1) * NB, :T2]
        z2n = np.asarray(results[c]["z2n"], np.float32)
        z[c * NB : (c + 1) * NB, T2:] = (
            z2n.reshape(4, T2, NCHUNK, 2).transpose(2, 0, 1, 3).reshape(NB, T2, 2)
        )
        ld[c * NB : (c + 1) * NB] = np.asarray(results[c]["ld"], np.float32).reshape(
            NB
        )
    return z, ld


def run(inputs, trace=False, trace_cores=None):
    """Run on 8 NeuronCores; returns ((z, logdet), BassKernelResults)."""
    from concourse.bass_utils import run_bass_kernel_spmd

    nc = _get_program()
    in_maps = _host_inputs(inputs)
    res = run_bass_kernel_spmd(
        nc,
        in_maps,
        list(range(NCORES)),
        trace=trace,
        trace_cores=trace_cores if trace_cores is not None else list(range(NCORES)),
    )
    x = np.asarray(inputs["x"], np.float32)
    return _assemble_output(x, res.results), res


def kernel(**inputs):
    (z, ld), _ = run(inputs, trace=False)
    return z, ld


if __name__ == "__main__":
    print("build only:", _get_program())


# revision 50
# speedup vs baseline: 1.1101x; 1.0068x over previous
"""Trainium2 Bass kernel for nn_NlsqCond (ConvFlow NLSQ coupling layer).

Strategy: pure data parallel over batch B=256 -> 32 samples per core on 8
NeuronCores. Convs are computed as 3 shifted matmuls accumulating in PSUM
over a gap-column activation layout (34 columns per sample, zero guard
columns), so the k=3/pad=1 conv needs no boundary special-casing. Weights
are transposed host-side into lhsT layout and cast to bf16 (fp32 PSUM
accumulation); measured end-to-end error vs fp32 reference is ~1e-4.
The final projection is computed transposed ([cols, 10]) so the NLSQ
elementwise tail runs with full 128-partition parallelism; the per-sample
logdet partition-reduction is done with a small mask matmul.
"""

import math

import numpy as np

B, T, D, H, COND = 256, 64, 2, 512, 8
T2 = T // 2                      # 32
NCORES = 8
NB = B // NCORES                 # 32 samples per core
ST = T2 + 2                      # 34: per-sample column stride (zero gaps)
WCOLS = NB * ST                  # 1088
NG = 2                           # PSUM column groups per matmul set
GS = NB // NG                    # 16 samples per group
NCHUNK = NB * T2 // 128          # 8 column chunks of 128 for final proj
LOG_A = math.log(8.0 * math.sqrt(3.0) / 9.0 - 0.05)

_CACHE = {}


def _build_program():
    import concourse.bacc as bacc
    import concourse.mybir as mybir
    import concourse.tile as tile

    f32 = mybir.dt.float32
    bf16 = mybir.dt.bfloat16
    AF = mybir.ActivationFunctionType
    Alu = mybir.AluOpType

    nc = bacc.Bacc("TRN2", target_bir_lowering=False, debug=False)

    # ---- DRAM I/O ----
    # z1 in gap layout (zero guard columns), rows replicated for the 3 conv
    # shifts: row (k*2+d) col m = z1_gap[d, m+k]. Feeds conv0's h-half as a
    # single K=6 matmul per window.
    d_z1g = nc.dram_tensor("z1g", [32, WCOLS], bf16, kind="ExternalInput")
    d_z2r = nc.dram_tensor("z2r", [128, NCHUNK, 2], f32, kind="ExternalInput")
    d_condT = nc.dram_tensor("condT", [128, NB], bf16, kind="ExternalInput")
    d_wc1 = nc.dram_tensor("wc1", [128, H], bf16, kind="ExternalInput")
    # conv/linear weights packed partition-major so each DMA moves one large
    # contiguous chunk per partition (descriptor-rate, not bandwidth, limits
    # small-row DMAs)
    d_wc2 = nc.dram_tensor("wc2", [128, 4, H], bf16, kind="ExternalInput")
    # conv0 h-half collapsed to rank 6: wh6[k*2+d] = (w_conv0[:, :H, k] @ we).T
    d_wh6 = nc.dram_tensor("wh6", [128, H], bf16, kind="ExternalInput")
    # conv0 cond-half collapsed to per-sample vectors: types (sum_k, k=0, k=2)
    d_qw = nc.dram_tensor("qw", [128, 4, 3, H], bf16, kind="ExternalInput")
    # b_embed fold rank-1 terms per type
    d_vb = nc.dram_tensor("vb", [1, 3, H], bf16, kind="ExternalInput")
    # signed indicator matrix applying q to the conv0 windows on the PE:
    # row ty*32+s, col m -> coefficient of q[ty][s] in output position m+1
    d_ik = nc.dram_tensor("ik", [128, WCOLS], bf16, kind="ExternalInput")
    d_wcv = nc.dram_tensor("wcv", [4, 128, 4, 3, H], bf16, kind="ExternalInput")
    d_wout = nc.dram_tensor("wout", [128, 4, 10], bf16, kind="ExternalInput")
    d_bias = nc.dram_tensor("biases", [128, 8, 4], f32, kind="ExternalInput")
    d_brep = nc.dram_tensor("brep", [128, NCHUNK, 10], f32, kind="ExternalInput")
    d_mask = nc.dram_tensor("mask", [128, 4], f32, kind="ExternalInput")
    d_z2n = nc.dram_tensor("z2n", [128, NCHUNK, 2], f32, kind="ExternalOutput")
    d_ld = nc.dram_tensor("ld", [NCHUNK, 4], f32, kind="ExternalOutput")

    with tile.TileContext(nc) as tc:
        with (
            tc.tile_pool(name="w", bufs=1) as wp,
            tc.tile_pool(name="act", bufs=1) as ap_,
            tc.tile_pool(name="ew", bufs=1) as ewp,
            tc.tile_pool(name="ps", bufs=8, space="PSUM") as pp,
        ):
            def mk(pool, shape, dtype, tag):
                return pool.tile(shape, dtype, tag=tag, name=tag)

            # ---- SBUF loads ----
            # issue DMAs from several engine queues in parallel (each
            # DMA_DIRECT2D costs ~0.6-1.3us of serial issue time per queue)
            z1g_sb = mk(ap_, [128, WCOLS], bf16, "z1g")
            nc.gpsimd.dma_start(z1g_sb[:32], d_z1g.ap())
            for pb in (32, 64, 96):
                nc.vector.memset(z1g_sb[pb : pb + 32, :], 0.0)
            wh6_sb = mk(wp, [128, H], bf16, "wh6")
            nc.gpsimd.dma_start(wh6_sb[:], d_wh6.ap())
            condT_sb = mk(ap_, [128, NB], bf16, "condT")
            nc.gpsimd.dma_start(condT_sb[:], d_condT.ap())
            wc1_sb = mk(wp, [128, H], bf16, "wc1")
            nc.gpsimd.dma_start(wc1_sb[:], d_wc1.ap())
            bias_sb = mk(wp, [128, 8, 4], f32, "bias")
            nc.gpsimd.dma_start(bias_sb[:], d_bias.ap())
            wc2_sb = mk(wp, [128, 4, H], bf16, "wc2")
            nc.sync.dma_start(wc2_sb[:], d_wc2.ap())
            vb_sb = mk(wp, [1, 3, H], bf16, "vb")
            nc.sync.dma_start(vb_sb[:], d_vb.ap())
            qw_sb = mk(wp, [128, 4, 3, H], bf16, "qw")
            for kt in range(4):
                nc.sync.dma_start(qw_sb[:, kt], d_qw.ap()[:, kt])
            ik_sb = mk(wp, [128, WCOLS], bf16, "ik")
            nc.sync.dma_start(ik_sb[:96], d_ik.ap()[:96])
            nc.vector.memset(ik_sb[96:128, :], 0.0)
            wcv_sb = [mk(wp, [128, 4, 3, H], bf16, f"wcv_{l}") for l in range(4)]
            for l in range(4):
                nc.sync.dma_start(wcv_sb[l][:, 0:2], d_wcv.ap()[l, :, 0:2])
                nc.sync.dma_start(wcv_sb[l][:, 2:4], d_wcv.ap()[l, :, 2:4])
            wout_sb = mk(wp, [128, 4, 10], bf16, "wout")
            nc.sync.dma_start(wout_sb[:], d_wout.ap())
            brep_sb = mk(wp, [128, NCHUNK, 10], f32, "brep")
            nc.sync.dma_start(brep_sb[:], d_brep.ap())
            mask_sb = mk(wp, [128, 4], f32, "mask")
            nc.sync.dma_start(mask_sb[:], d_mask.ap())
            z2r_sb = mk(ewp, [128, NCHUNK, 2], f32, "z2r")
            nc.sync.dma_start(z2r_sb[:], d_z2r.ap())

            def bias_ap(idx, mt):
                return bias_sb[:, idx, mt : mt + 1]

            # ---- PE warm-up scratch (memset first so the PE can spin ASAP) ----
            wu_l = mk(ap_, [128, 128], bf16, "wu_l")
            wu_r = mk(ap_, [128, 512], bf16, "wu_r")
            nc.vector.memset(wu_l[:], 0.0)
            nc.vector.memset(wu_r[:], 0.0)

            # ---- activation ping-pong buffers (gap layout) ----
            # Only the gap guard columns need zeroing; valid columns are
            # always written before they are read.
            actB = [mk(ap_, [128, WCOLS], bf16, f"B{i}") for i in range(4)]
            actC = [mk(ap_, [128, WCOLS], bf16, f"C{i}") for i in range(4)]
            for ti, t_ in enumerate(actB + actC):
                v = t_[:].rearrange("p (s c) -> p s c", c=ST)
                eng = nc.vector if ti % 2 else nc.gpsimd
                eng.memset(v[:, :, 0:1], 0.0)
                eng.memset(v[:, :, ST - 1 : ST], 0.0)
            ones_sb = mk(ap_, [1, NB], bf16, "ones")
            nc.vector.memset(ones_sb[:], 1.0)

            def valid(tl, g=None):
                v = tl[:].rearrange("p (s c) -> p s c", c=ST)
                if g is None:
                    return v[:, :, 1 : 1 + T2]
                return v[:, g * GS : (g + 1) * GS, 1 : 1 + T2]

            # ---- PE warm-up spin ----
            # The PE HAM clock gate starts at 1.2 GHz and only releases to
            # 2.4 GHz after ~3.4us of sustained activity. Matmul on scratch
            # zeros while the weight DMAs stream in, so the real conv stack
            # runs warm from its first instruction.
            ps_w = mk(pp, [128, 512], f32, "ps")
            for i in range(10):
                nc.tensor.matmul(
                    ps_w[:], wu_l[:], wu_r[:], start=(i == 0), stop=(i == 9)
                )

            # preload the ACT transcendental tables so the elementwise tail
            # doesn't pay the table-swap latency
            scr = mk(ewp, [1, 4], f32, "scr")
            nc.scalar.activation(scr[:, 0:1], wu_l[:1, 0:1], AF.Exp)
            nc.scalar.activation(scr[:, 1:2], wu_l[:1, 0:1], AF.Tanh)
            nc.scalar.activation(scr[:, 2:3], wu_l[:1, 0:1], AF.Ln, bias=1.0)

            # ---- cond MLP: c2 = relu(W2 relu(W1 c + b1) + b2) ----
            c1_sb = [mk(ap_, [128, NB], bf16, f"c1_{i}") for i in range(4)]
            c2_sb = [mk(ap_, [128, NB], bf16, f"c2_{i}") for i in range(4)]
            for mt in range(4):
                ps = mk(pp, [128, 512], f32, "ps")
                nc.tensor.matmul(
                    ps[:, :NB],
                    wc1_sb[:, mt * 128 : (mt + 1) * 128],
                    condT_sb[:],
                    start=True,
                    stop=True,
                )
                nc.scalar.activation(
                    c1_sb[mt][:], ps[:, :NB], AF.Relu, bias=bias_ap(1, mt)
                )
            for mt in range(4):
                ps = mk(pp, [128, 512], f32, "ps")
                for kt in range(4):
                    nc.tensor.matmul(
                        ps[:, :NB],
                        wc2_sb[:, kt, mt * 128 : (mt + 1) * 128],
                        c1_sb[kt][:],
                        start=(kt == 0),
                        stop=(kt == 3),
                    )
                nc.scalar.activation(
                    c2_sb[mt][:], ps[:, :NB], AF.Relu, bias=bias_ap(2, mt)
                )

            WINDOWS = [(0, 15), (15, 15), (30, 2)]  # (sample base, n samples)

            def relu_copy(L, mt, wi, pv_, dsts, sb, ns):
                dv_ = dsts[mt][:].rearrange("p (s c) -> p s c", c=ST)[
                    :, sb : sb + ns, 1 : 1 + T2
                ]
                # split the PSUM->SBUF relu copies across ACT and DVE
                if (mt * 3 + wi) % 2 == 0:
                    nc.scalar.activation(dv_, pv_, AF.Relu, bias=bias_ap(3 + L, mt))
                else:
                    nc.vector.tensor_scalar(
                        dv_, pv_, bias_ap(3 + L, mt), 0.0, Alu.add, Alu.max
                    )

            # ---- conv0 ----
            # qT[ty*32+s, o]: per-sample cond/bias vectors, computed
            # transposed so they can be applied to the conv0 windows by a
            # single K=96 indicator matmul inside the PSUM accumulation:
            #   ty=0: (sum_k W0k_c) @ c2[s] + sum_k(W0k_h @ b_embed)
            #   ty=1: W00_c @ c2[s] + v0   (subtracted at t=0)
            #   ty=2: W02_c @ c2[s] + v2   (subtracted at t=31)
            psq = mk(pp, [96, 512], f32, "ps")
            for ty in range(3):
                for kt in range(4):
                    nc.tensor.matmul(
                        psq[ty * NB : (ty + 1) * NB, :],
                        c2_sb[kt][:],
                        qw_sb[:, kt, ty, :],
                        start=(kt == 0),
                        stop=False,
                    )
                nc.tensor.matmul(
                    psq[ty * NB : (ty + 1) * NB, :],
                    ones_sb[:],
                    vb_sb[:, ty, :],
                    start=False,
                    stop=True,
                )
            qT_sb = mk(ap_, [128, H], bf16, "qT")
            nc.vector.memset(qT_sb[96:128, :], 0.0)
            nc.vector.tensor_copy(qT_sb[:96, :], psq[:])

            # h-half: single K=6 matmul per window (the 3 conv shifts are
            # baked into the replicated z1g rows); q applied via indicator
            for mt in range(4):
                pss = [mk(pp, [128, 512], f32, "ps") for _ in WINDOWS]
                for wi, (sb, ns) in enumerate(WINDOWS):
                    n = ns * ST - 2
                    base = sb * ST
                    nc.tensor.matmul(
                        pss[wi][:, :n],
                        wh6_sb[:, mt * 128 : (mt + 1) * 128],
                        z1g_sb[:, base : base + n],
                        start=True,
                        stop=False,
                    )
                    nc.tensor.matmul(
                        pss[wi][:, :n],
                        qT_sb[:, mt * 128 : (mt + 1) * 128],
                        ik_sb[:, base : base + n],
                        start=False,
                        stop=True,
                    )
                for wi, (sb, ns) in enumerate(WINDOWS):
                    pv3 = pss[wi][:, : ns * ST].rearrange("p (s c) -> p s c", c=ST)
                    relu_copy(0, mt, wi, pv3[:, :, 0:T2], actB, sb, ns)

            # ---- conv1..4 ----
            # Matmul moving operands must be single-free-dim, so each conv
            # matmul streams a contiguous window of the gap layout; outputs
            # at gap positions are garbage and simply never read back.
            for L in range(1, 5):
                srcs = [actB, actC, actB, actC][L - 1]
                wts = wcv_sb[L - 1][:]
                dsts = actB if L % 2 == 0 else actC
                for mt in range(4):
                    pss = [mk(pp, [128, 512], f32, "ps") for _ in WINDOWS]
                    nacc = 12
                    i = 0
                    # kt outer: matches weight-DMA arrival order
                    for kt in range(4):
                        for k in range(3):
                            lhsT = wts[:, kt, k, mt * 128 : (mt + 1) * 128]
                            for wi, (sb, ns) in enumerate(WINDOWS):
                                n = ns * ST - 2
                                base = sb * ST + k
                                nc.tensor.matmul(
                                    pss[wi][:, :n],
                                    lhsT,
                                    srcs[kt][:, base : base + n],
                                    start=(i == 0),
                                    stop=(i == nacc - 1),
                                )
                            i += 1
                    for wi, (sb, ns) in enumerate(WINDOWS):
                        pv3 = pss[wi][:, : ns * ST].rearrange(
                            "p (s c) -> p s c", c=ST
                        )
                        relu_copy(L, mt, wi, pv3[:, :, 0:T2], dsts, sb, ns)
            srcs = actB

            # ---- compact the final activation (drop gap columns) ----
            h5 = [mk(ap_, [128, NB * T2], bf16, f"h5_{i}") for i in range(4)]
            for kt in range(4):
                eng = nc.scalar if kt % 2 == 0 else nc.vector
                if kt % 2 == 0:
                    nc.scalar.copy(
                        h5[kt][:].rearrange("p (s t) -> p s t", t=T2),
                        valid(srcs[kt]),
                    )
                else:
                    nc.vector.tensor_copy(
                        h5[kt][:].rearrange("p (s t) -> p s t", t=T2),
                        valid(srcs[kt]),
                    )

            # ---- final projection (transposed): out[col, 10] ----
            pso = mk(pp, [128, 512], f32, "ps")
            for j in range(NCHUNK):
                for kt in range(4):
                    nc.tensor.matmul(
                        pso[:, j * 10 : (j + 1) * 10],
                        h5[kt][:, j * 128 : (j + 1) * 128],
                        wout_sb[:, kt],
                        start=(kt == 0),
                        stop=(kt == 3),
                    )

            # ---- NLSQ elementwise tail ----
            def ew(tag):
                return mk(ewp, [128, NCHUNK, 2], f32, tag)

            params = mk(ewp, [128, NCHUNK, 10], f32, "params")
            nc.vector.tensor_add(
                params[:],
                pso[:, : NCHUNK * 10].rearrange("p (j q) -> p j q", q=10),
                brep_sb[:],
            )
            pv = params[:].rearrange("p j (a q) -> p j a q", q=5)
            P0, P1, P2, P3, P4 = (pv[:, :, :, i] for i in range(5))

            loga_sb = mk(ewp, [128, 1], f32, "loga")
            nc.vector.memset(loga_sb[:], LOG_A)

            # group ACT functions (Exp x3, then Tanh, Ln last) to minimize
            # activation-table reloads
            u = ew("u")
            nc.vector.tensor_tensor(u[:], P1, P3, op=Alu.subtract)
            b_ = ew("b_")
            nc.scalar.activation(b_[:], P1, AF.Exp, scale=0.4)
            d_ = ew("d_")
            nc.scalar.activation(d_[:], P3, AF.Exp, scale=0.4)
            e = ew("e")
            nc.scalar.activation(e[:], u[:], AF.Exp, scale=0.4, bias=loga_sb[:])
            th = ew("th")
            nc.scalar.activation(th[:], P2, AF.Tanh, scale=0.3)
            c_ = ew("c_")
            nc.vector.tensor_mul(c_[:], th[:], e[:])
            t1 = ew("t1")
            nc.vector.tensor_mul(t1[:], d_[:], z2r_sb[:])
            arg = ew("arg")
            nc.vector.tensor_add(arg[:], t1[:], P4)
            sq = ew("sq")
            nc.vector.tensor_mul(sq[:], arg[:], arg[:])
            den = ew("den")
            nc.vector.tensor_scalar_add(den[:], sq[:], 1.0)
            rcp = ew("rcp")
            nc.vector.reciprocal(rcp[:], den[:])
            t2 = ew("t2")
            nc.vector.tensor_mul(t2[:], b_[:], z2r_sb[:])
            t3 = ew("t3")
            nc.vector.tensor_mul(t3[:], c_[:], rcp[:])
            s1 = ew("s1")
            nc.vector.tensor_add(s1[:], P0, t2[:])
            z2n_sb = ew("z2n_sb")
            nc.vector.tensor_add(z2n_sb[:], s1[:], t3[:])
            nc.sync.dma_start(d_z2n.ap(), z2n_sb[:])

            t4 = ew("t4")
            nc.vector.tensor_mul(t4[:], c_[:], d_[:])
            t5 = ew("t5")
            nc.vector.tensor_mul(t5[:], t4[:], arg[:])
            t6 = ew("t6")
            nc.vector.tensor_mul(t6[:], t5[:], rcp[:])
            t7 = ew("t7")
            nc.vector.tensor_mul(t7[:], t6[:], rcp[:])
            inner = ew("inner")
            nc.vector.scalar_tensor_tensor(
                inner[:], t7[:], -2.0, b_[:], op0=Alu.mult, op1=Alu.add
            )
            lg = ew("lg")
            nc.scalar.activation(lg[:], inner[:], AF.Ln)

            lg2 = mk(ewp, [128, NCHUNK], f32, "lg2")
            nc.vector.tensor_add(lg2[:], lg[:, :, 0], lg[:, :, 1])
            psl = mk(pp, [128, 512], f32, "ps")
            nc.tensor.matmul(
                psl[:NCHUNK, :4], lg2[:], mask_sb[:], start=True, stop=True
            )
            ld_sb = mk(ewp, [NCHUNK, 4], f32, "ld_sb")
            nc.vector.tensor_copy(ld_sb[:], psl[:NCHUNK, :4])
            nc.sync.dma_start(d_ld.ap(), ld_sb[:])

    nc.compile()
    return nc


def _get_program():
    if "nc" not in _CACHE:
        _CACHE["nc"] = _build_program()
    return _CACHE["nc"]


def _host_inputs(inputs):
    import ml_dtypes

    bf16 = ml_dtypes.bfloat16
    f32 = np.float32

    x = np.asarray(inputs["x"], f32)
    cond = np.asarray(inputs["cond"], f32)

    wc1T = np.zeros((128, H), f32)
    wc1T[: 2 * COND] = inputs["w_c1"].T
    wc1T = wc1T.astype(bf16)
    # partition-major packing: [128, ...] with large contiguous per-partition
    # chunks so SBUF DMAs are few big descriptors per partition
    wc2T = np.ascontiguousarray(
        inputs["w_c2"].T.reshape(4, 128, H).transpose(1, 0, 2)
    ).astype(bf16)
    # conv0 h-half collapsed through the rank-2 embedding; rows (k*2+d)
    w0 = np.asarray(inputs["w_conv0"], f32)
    we2 = np.asarray(inputs["w_embed"], f32)[:, :2]
    wh6 = np.zeros((128, H), f32)
    wh6[:6] = np.einsum("ock,cd->kdo", w0[:, :H, :], we2).reshape(6, H)
    wh6 = wh6.astype(bf16)
    # conv0 cond-half collapsed to per-sample vectors (types: sum_k, k0, k2)
    S = w0[:, H:, :]
    mats = np.stack([(S[:, :, 0] + S[:, :, 1] + S[:, :, 2]).T, S[:, :, 0].T, S[:, :, 2].T])
    qw = np.ascontiguousarray(
        mats.reshape(3, 4, 128, H).transpose(2, 1, 0, 3)
    ).astype(bf16)
    # b_embed folded through conv0 (rank-1 terms per type)
    be = np.asarray(inputs["b_embed"], f32)
    v = np.stack([w0[:, :H, k] @ be for k in range(3)])  # [3, H(out)]
    vb = np.ascontiguousarray(
        np.stack([v[0] + v[1] + v[2], v[0], v[2]])[None, :, :]
    ).astype(bf16)
    wcv = np.stack(
        [
            np.ascontiguousarray(
                np.transpose(inputs[f"w_conv{i}"], (1, 2, 0))
                .reshape(4, 128, 3, H)
                .transpose(1, 0, 2, 3)
            )
            for i in (1, 2, 3, 4)
        ]
    ).astype(bf16)
    woutT = np.ascontiguousarray(
        inputs["w_out"].T.reshape(4, 128, 10).transpose(1, 0, 2)
    ).astype(bf16)
    bias_all = np.stack(
        [inputs["b_embed"], inputs["b_c1"], inputs["b_c2"]]
        + [inputs[f"b_conv{i}"] for i in range(5)]
    ).astype(f32)
    bias_pack = np.ascontiguousarray(
        bias_all.reshape(8, 4, 128).transpose(2, 0, 1)
    ).astype(f32)
    brep = np.ascontiguousarray(
        np.broadcast_to(inputs["b_out"].astype(f32), (128, NCHUNK, 10))
    )
    mask = np.zeros((128, 4), f32)
    mask[np.arange(128), np.arange(128) // 32] = 1.0
    ik = np.zeros((128, WCOLS), f32)
    for s in range(NB):
        ik[s, ST * s : ST * s + T2] = 1.0
        ik[NB + s, ST * s] = -1.0
        ik[2 * NB + s, ST * s + T2 - 1] = -1.0
    ik = ik.astype(bf16)

    in_maps = []
    for c in range(NCORES):
        xs = x[c * NB : (c + 1) * NB]
        z1 = xs[:, :T2]
        z2 = xs[:, T2:]
        z1p = np.zeros((2, WCOLS + 2), np.float32)
        z1p[:, :WCOLS].reshape(2, NB, ST)[:, :, 1 : 1 + T2] = z1.transpose(2, 0, 1)
        z1g = np.zeros((32, WCOLS), np.float32)
        z1g[:6] = np.stack(
            [z1p[dd, k : k + WCOLS] for k in range(3) for dd in range(2)]
        )
        z1g = z1g.astype(bf16)
        z2r = np.ascontiguousarray(
            z2.reshape(NCHUNK, 4, T2, 2).transpose(1, 2, 0, 3)
        ).reshape(128, NCHUNK, 2)
        condT = np.zeros((128, NB), np.float32)
        condT[: 2 * COND] = cond[c * NB : (c + 1) * NB].reshape(NB, 2 * COND).T
        condT = condT.astype(bf16)
        in_maps.append(
            dict(
                z1g=z1g,
                z2r=z2r,
                condT=condT,
                wc1=wc1T,
                wc2=wc2T,
                wh6=wh6,
                qw=qw,
                vb=vb,
                ik=ik,
                wcv=wcv,
                wout=woutT,
                biases=bias_pack,
                brep=brep,
                mask=mask,
            )
        )
    return in_maps


def _assemble_output(x, results):
    z = np.empty((B, T, D), np.float32)
    ld = np.empty((B,), np.float32)
    for c in range(NCORES):
        z[c * NB : (c + 1) * NB, :T2] = x[c * NB : (c + 1) * NB, :T2]
        z2n = np.asarray(results[c]["z2n"], np.float32)
        z[c * NB : (c + 1) * NB, T2:] = (
            z2n.reshape(4, T2, NCHUNK, 2).transpose(2, 0, 1, 3).reshape(NB, T2, 2)
        )
        ld[c * NB : (c + 1) * NB] = np.asarray(results[c]["ld"], np.float32).reshape(
            NB
        )
    return z, ld


def run(inputs, trace=False, trace_cores=None):
    """Run on 8 NeuronCores; returns ((z, logdet), BassKernelResults)."""
    from concourse.bass_utils import run_bass_kernel_spmd

    nc = _get_program()
    in_maps = _host_inputs(inputs)
    res = run_bass_kernel_spmd(
        nc,
        in_maps,
        list(range(NCORES)),
        trace=trace,
        trace_cores=trace_cores if trace_cores is not None else list(range(NCORES)),
    )
    x = np.asarray(inputs["x"], np.float32)
    return _assemble_output(x, res.results), res


def kernel(**inputs):
    (z, ld), _ = run(inputs, trace=False)
    return z, ld


if __name__ == "__main__":
    print("build only:", _get_program())


# revision 51
# speedup vs baseline: 1.1299x; 1.0179x over previous
"""Trainium2 Bass kernel for nn_NlsqCond (ConvFlow NLSQ coupling layer).

Strategy: pure data parallel over batch B=256 -> 32 samples per core on 8
NeuronCores. Convs are computed as 3 shifted matmuls accumulating in PSUM
over a gap-column activation layout (34 columns per sample, zero guard
columns), so the k=3/pad=1 conv needs no boundary special-casing. Weights
are transposed host-side into lhsT layout and cast to bf16 (fp32 PSUM
accumulation); measured end-to-end error vs fp32 reference is ~1e-4.
The final projection is computed transposed ([cols, 10]) so the NLSQ
elementwise tail runs with full 128-partition parallelism; the per-sample
logdet partition-reduction is done with a small mask matmul.
"""

import math

import numpy as np

B, T, D, H, COND = 256, 64, 2, 512, 8
T2 = T // 2                      # 32
NCORES = 8
NB = B // NCORES                 # 32 samples per core
ST = T2 + 2                      # 34: per-sample column stride (zero gaps)
WCOLS = NB * ST                  # 1088
NG = 2                           # PSUM column groups per matmul set
GS = NB // NG                    # 16 samples per group
NCHUNK = NB * T2 // 128          # 8 column chunks of 128 for final proj
LOG_A = math.log(8.0 * math.sqrt(3.0) / 9.0 - 0.05)

_CACHE = {}


def _build_program():
    import concourse.bacc as bacc
    import concourse.mybir as mybir
    import concourse.tile as tile

    f32 = mybir.dt.float32
    bf16 = mybir.dt.bfloat16
    AF = mybir.ActivationFunctionType
    Alu = mybir.AluOpType

    nc = bacc.Bacc("TRN2", target_bir_lowering=False, debug=False)

    # ---- DRAM I/O ----
    # z1 in gap layout (zero guard columns), rows replicated for the 3 conv
    # shifts: row (k*2+d) col m = z1_gap[d, m+k]. Feeds conv0's h-half as a
    # single K=6 matmul per window.
    d_z1g = nc.dram_tensor("z1g", [32, WCOLS], bf16, kind="ExternalInput")
    d_z2r = nc.dram_tensor("z2r", [128, NCHUNK, 2], f32, kind="ExternalInput")
    d_condT = nc.dram_tensor("condT", [128, NB], bf16, kind="ExternalInput")
    d_wc1 = nc.dram_tensor("wc1", [128, H], bf16, kind="ExternalInput")
    # conv/linear weights packed partition-major so each DMA moves one large
    # contiguous chunk per partition (descriptor-rate, not bandwidth, limits
    # small-row DMAs)
    d_wc2 = nc.dram_tensor("wc2", [128, 4, H], bf16, kind="ExternalInput")
    # conv0 h-half collapsed to rank 6: wh6[k*2+d] = (w_conv0[:, :H, k] @ we).T
    d_wh6 = nc.dram_tensor("wh6", [128, H], bf16, kind="ExternalInput")
    # conv0 cond-half collapsed to per-sample vectors: types (sum_k, k=0, k=2)
    d_qw = nc.dram_tensor("qw", [128, 4, 3, H], bf16, kind="ExternalInput")
    # b_embed fold rank-1 terms per type
    d_vb = nc.dram_tensor("vb", [1, 3, H], bf16, kind="ExternalInput")
    # signed indicator matrix applying q to the conv0 windows on the PE:
    # row ty*32+s, col m -> coefficient of q[ty][s] in output position m+1
    d_ik = nc.dram_tensor("ik", [128, WCOLS], bf16, kind="ExternalInput")
    d_wcv = nc.dram_tensor("wcv", [4, 128, 4, 3, H], bf16, kind="ExternalInput")
    d_wout = nc.dram_tensor("wout", [128, 4, 10], bf16, kind="ExternalInput")
    d_bias = nc.dram_tensor("biases", [128, 8, 4], f32, kind="ExternalInput")
    d_brep = nc.dram_tensor("brep", [128, NCHUNK, 10], f32, kind="ExternalInput")
    d_mask = nc.dram_tensor("mask", [128, 4], f32, kind="ExternalInput")
    d_z2n = nc.dram_tensor("z2n", [128, NCHUNK, 2], f32, kind="ExternalOutput")
    d_ld = nc.dram_tensor("ld", [NCHUNK, 4], f32, kind="ExternalOutput")

    with tile.TileContext(nc) as tc:
        with (
            tc.tile_pool(name="w", bufs=1) as wp,
            tc.tile_pool(name="act", bufs=1) as ap_,
            tc.tile_pool(name="ew", bufs=1) as ewp,
            tc.tile_pool(name="ps", bufs=8, space="PSUM") as pp,
        ):
            def mk(pool, shape, dtype, tag):
                return pool.tile(shape, dtype, tag=tag, name=tag)

            # ---- SBUF loads ----
            # issue DMAs from several engine queues in parallel (each
            # DMA_DIRECT2D costs ~0.6-1.3us of serial issue time per queue)
            z1g_sb = mk(ap_, [128, WCOLS], bf16, "z1g")
            nc.gpsimd.dma_start(z1g_sb[:32], d_z1g.ap())
            for pb in (32, 64, 96):
                nc.vector.memset(z1g_sb[pb : pb + 32, :], 0.0)
            wh6_sb = mk(wp, [128, H], bf16, "wh6")
            nc.gpsimd.dma_start(wh6_sb[:], d_wh6.ap())
            condT_sb = mk(ap_, [128, NB], bf16, "condT")
            nc.gpsimd.dma_start(condT_sb[:], d_condT.ap())
            wc1_sb = mk(wp, [128, H], bf16, "wc1")
            nc.gpsimd.dma_start(wc1_sb[:], d_wc1.ap())
            bias_sb = mk(wp, [128, 8, 4], f32, "bias")
            nc.gpsimd.dma_start(bias_sb[:], d_bias.ap())
            wc2_sb = mk(wp, [128, 4, H], bf16, "wc2")
            nc.sync.dma_start(wc2_sb[:], d_wc2.ap())
            vb_sb = mk(wp, [1, 3, H], bf16, "vb")
            nc.sync.dma_start(vb_sb[:], d_vb.ap())
            qw_sb = mk(wp, [128, 4, 3, H], bf16, "qw")
            for kt in range(4):
                nc.sync.dma_start(qw_sb[:, kt], d_qw.ap()[:, kt])
            ik_sb = mk(wp, [128, WCOLS], bf16, "ik")
            nc.sync.dma_start(ik_sb[:96], d_ik.ap()[:96])
            nc.vector.memset(ik_sb[96:128, :], 0.0)
            wcv_sb = [mk(wp, [128, 4, 3, H], bf16, f"wcv_{l}") for l in range(4)]
            for l in range(4):
                nc.sync.dma_start(wcv_sb[l][:, 0:2], d_wcv.ap()[l, :, 0:2])
                nc.sync.dma_start(wcv_sb[l][:, 2:4], d_wcv.ap()[l, :, 2:4])
            wout_sb = mk(wp, [128, 4, 10], bf16, "wout")
            nc.sync.dma_start(wout_sb[:], d_wout.ap())
            brep_sb = mk(wp, [128, NCHUNK, 10], f32, "brep")
            nc.sync.dma_start(brep_sb[:], d_brep.ap())
            mask_sb = mk(wp, [128, 4], f32, "mask")
            nc.sync.dma_start(mask_sb[:], d_mask.ap())
            z2r_sb = mk(ewp, [128, NCHUNK, 2], f32, "z2r")
            nc.sync.dma_start(z2r_sb[:], d_z2r.ap())

            def bias_ap(idx, mt):
                return bias_sb[:, idx, mt : mt + 1]

            # ---- PE warm-up scratch (memset first so the PE can spin ASAP) ----
            wu_l = mk(ap_, [128, 128], bf16, "wu_l")
            wu_r = mk(ap_, [128, 512], bf16, "wu_r")
            nc.vector.memset(wu_l[:], 0.0)
            nc.vector.memset(wu_r[:], 0.0)

            # ---- activation ping-pong buffers (gap layout) ----
            # Only the gap guard columns need zeroing; valid columns are
            # always written before they are read.
            actB = [mk(ap_, [128, WCOLS], bf16, f"B{i}") for i in range(4)]
            actC = [mk(ap_, [128, WCOLS], bf16, f"C{i}") for i in range(4)]
            for ti, t_ in enumerate(actB + actC):
                v = t_[:].rearrange("p (s c) -> p s c", c=ST)
                eng = nc.vector if ti % 2 else nc.gpsimd
                eng.memset(v[:, :, 0:1], 0.0)
                eng.memset(v[:, :, ST - 1 : ST], 0.0)
            ones_sb = mk(ap_, [1, NB], bf16, "ones")
            nc.vector.memset(ones_sb[:], 1.0)

            def valid(tl, g=None):
                v = tl[:].rearrange("p (s c) -> p s c", c=ST)
                if g is None:
                    return v[:, :, 1 : 1 + T2]
                return v[:, g * GS : (g + 1) * GS, 1 : 1 + T2]

            # ---- PE warm-up spin ----
            # The PE HAM clock gate starts at 1.2 GHz and only releases to
            # 2.4 GHz after ~3.4us of sustained activity. Matmul on scratch
            # zeros while the weight DMAs stream in, so the real conv stack
            # runs warm from its first instruction.
            ps_w = mk(pp, [128, 512], f32, "ps")
            for i in range(10):
                nc.tensor.matmul(
                    ps_w[:], wu_l[:], wu_r[:], start=(i == 0), stop=(i == 9)
                )

            # preload the ACT transcendental tables so the elementwise tail
            # doesn't pay the table-swap latency
            scr = mk(ewp, [1, 4], f32, "scr")
            nc.scalar.activation(scr[:, 0:1], wu_l[:1, 0:1], AF.Exp)
            nc.scalar.activation(scr[:, 1:2], wu_l[:1, 0:1], AF.Tanh)
            nc.scalar.activation(scr[:, 2:3], wu_l[:1, 0:1], AF.Ln, bias=1.0)

            # ---- cond MLP: c2 = relu(W2 relu(W1 c + b1) + b2) ----
            c1_sb = [mk(ap_, [128, NB], bf16, f"c1_{i}") for i in range(4)]
            c2_sb = [mk(ap_, [128, NB], bf16, f"c2_{i}") for i in range(4)]
            for mt in range(4):
                ps = mk(pp, [128, 512], f32, "ps")
                nc.tensor.matmul(
                    ps[:, :NB],
                    wc1_sb[:, mt * 128 : (mt + 1) * 128],
                    condT_sb[:],
                    start=True,
                    stop=True,
                )
                nc.scalar.activation(
                    c1_sb[mt][:], ps[:, :NB], AF.Relu, bias=bias_ap(1, mt)
                )
            for mt in range(4):
                ps = mk(pp, [128, 512], f32, "ps")
                for kt in range(4):
                    nc.tensor.matmul(
                        ps[:, :NB],
                        wc2_sb[:, kt, mt * 128 : (mt + 1) * 128],
                        c1_sb[kt][:],
                        start=(kt == 0),
                        stop=(kt == 3),
                    )
                nc.scalar.activation(
                    c2_sb[mt][:], ps[:, :NB], AF.Relu, bias=bias_ap(2, mt)
                )

            WINDOWS = [(0, 15), (15, 15), (30, 2)]  # (sample base, n samples)

            def relu_copy(L, mt, wi, pv_, dsts, sb, ns):
                dv_ = dsts[mt][:].rearrange("p (s c) -> p s c", c=ST)[
                    :, sb : sb + ns, 1 : 1 + T2
                ]
                # relu copies on DVE only: any ACT Relu evicts the
                # transcendental tables and the elementwise tail would pay
                # two ~1.3us table reloads
                nc.vector.tensor_scalar(
                    dv_, pv_, bias_ap(3 + L, mt), 0.0, Alu.add, Alu.max
                )

            # ---- conv0 ----
            # qT[ty*32+s, o]: per-sample cond/bias vectors, computed
            # transposed so they can be applied to the conv0 windows by a
            # single K=96 indicator matmul inside the PSUM accumulation:
            #   ty=0: (sum_k W0k_c) @ c2[s] + sum_k(W0k_h @ b_embed)
            #   ty=1: W00_c @ c2[s] + v0   (subtracted at t=0)
            #   ty=2: W02_c @ c2[s] + v2   (subtracted at t=31)
            psq = mk(pp, [96, 512], f32, "ps")
            for ty in range(3):
                for kt in range(4):
                    nc.tensor.matmul(
                        psq[ty * NB : (ty + 1) * NB, :],
                        c2_sb[kt][:],
                        qw_sb[:, kt, ty, :],
                        start=(kt == 0),
                        stop=False,
                    )
                nc.tensor.matmul(
                    psq[ty * NB : (ty + 1) * NB, :],
                    ones_sb[:],
                    vb_sb[:, ty, :],
                    start=False,
                    stop=True,
                )
            qT_sb = mk(ap_, [128, H], bf16, "qT")
            nc.vector.memset(qT_sb[96:128, :], 0.0)
            nc.vector.tensor_copy(qT_sb[:96, :], psq[:])

            # h-half: single K=6 matmul per window (the 3 conv shifts are
            # baked into the replicated z1g rows); q applied via indicator
            for mt in range(4):
                pss = [mk(pp, [128, 512], f32, "ps") for _ in WINDOWS]
                for wi, (sb, ns) in enumerate(WINDOWS):
                    n = ns * ST - 2
                    base = sb * ST
                    nc.tensor.matmul(
                        pss[wi][:, :n],
                        wh6_sb[:, mt * 128 : (mt + 1) * 128],
                        z1g_sb[:, base : base + n],
                        start=True,
                        stop=False,
                    )
                    nc.tensor.matmul(
                        pss[wi][:, :n],
                        qT_sb[:, mt * 128 : (mt + 1) * 128],
                        ik_sb[:, base : base + n],
                        start=False,
                        stop=True,
                    )
                for wi, (sb, ns) in enumerate(WINDOWS):
                    pv3 = pss[wi][:, : ns * ST].rearrange("p (s c) -> p s c", c=ST)
                    relu_copy(0, mt, wi, pv3[:, :, 0:T2], actB, sb, ns)

            # ---- conv1..4 ----
            # Matmul moving operands must be single-free-dim, so each conv
            # matmul streams a contiguous window of the gap layout; outputs
            # at gap positions are garbage and simply never read back.
            for L in range(1, 5):
                srcs = [actB, actC, actB, actC][L - 1]
                wts = wcv_sb[L - 1][:]
                dsts = actB if L % 2 == 0 else actC
                for mt in range(4):
                    pss = [mk(pp, [128, 512], f32, "ps") for _ in WINDOWS]
                    nacc = 12
                    i = 0
                    # kt outer: matches weight-DMA arrival order
                    for kt in range(4):
                        for k in range(3):
                            lhsT = wts[:, kt, k, mt * 128 : (mt + 1) * 128]
                            for wi, (sb, ns) in enumerate(WINDOWS):
                                n = ns * ST - 2
                                base = sb * ST + k
                                nc.tensor.matmul(
                                    pss[wi][:, :n],
                                    lhsT,
                                    srcs[kt][:, base : base + n],
                                    start=(i == 0),
                                    stop=(i == nacc - 1),
                                )
                            i += 1
                    for wi, (sb, ns) in enumerate(WINDOWS):
                        pv3 = pss[wi][:, : ns * ST].rearrange(
                            "p (s c) -> p s c", c=ST
                        )
                        relu_copy(L, mt, wi, pv3[:, :, 0:T2], dsts, sb, ns)
            srcs = actB

            # ---- compact the final activation (drop gap columns) ----
            h5 = [mk(ap_, [128, NB * T2], bf16, f"h5_{i}") for i in range(4)]
            for kt in range(4):
                eng = nc.scalar if kt % 2 == 0 else nc.vector
                if kt % 2 == 0:
                    nc.scalar.copy(
                        h5[kt][:].rearrange("p (s t) -> p s t", t=T2),
                        valid(srcs[kt]),
                    )
                else:
                    nc.vector.tensor_copy(
                        h5[kt][:].rearrange("p (s t) -> p s t", t=T2),
                        valid(srcs[kt]),
                    )

            # ---- final projection (transposed): out[col, 10] ----
            pso = mk(pp, [128, 512], f32, "ps")
            for j in range(NCHUNK):
                for kt in range(4):
                    nc.tensor.matmul(
                        pso[:, j * 10 : (j + 1) * 10],
                        h5[kt][:, j * 128 : (j + 1) * 128],
                        wout_sb[:, kt],
                        start=(kt == 0),
                        stop=(kt == 3),
                    )

            # ---- NLSQ elementwise tail ----
            def ew(tag):
                return mk(ewp, [128, NCHUNK, 2], f32, tag)

            params = mk(ewp, [128, NCHUNK, 10], f32, "params")
            nc.vector.tensor_add(
                params[:],
                pso[:, : NCHUNK * 10].rearrange("p (j q) -> p j q", q=10),
                brep_sb[:],
            )
            pv = params[:].rearrange("p j (a q) -> p j a q", q=5)
            P0, P1, P2, P3, P4 = (pv[:, :, :, i] for i in range(5))

            loga_sb = mk(ewp, [128, 1], f32, "loga")
            nc.vector.memset(loga_sb[:], LOG_A)

            # group ACT functions (Exp x3, then Tanh, Ln last) to minimize
            # activation-table reloads
            u = ew("u")
            nc.vector.tensor_tensor(u[:], P1, P3, op=Alu.subtract)
            b_ = ew("b_")
            nc.scalar.activation(b_[:], P1, AF.Exp, scale=0.4)
            d_ = ew("d_")
            nc.scalar.activation(d_[:], P3, AF.Exp, scale=0.4)
            e = ew("e")
            nc.scalar.activation(e[:], u[:], AF.Exp, scale=0.4, bias=loga_sb[:])
            th = ew("th")
            nc.scalar.activation(th[:], P2, AF.Tanh, scale=0.3)
            c_ = ew("c_")
            nc.vector.tensor_mul(c_[:], th[:], e[:])
            t1 = ew("t1")
            nc.vector.tensor_mul(t1[:], d_[:], z2r_sb[:])
            arg = ew("arg")
            nc.vector.tensor_add(arg[:], t1[:], P4)
            sq = ew("sq")
            nc.vector.tensor_mul(sq[:], arg[:], arg[:])
            den = ew("den")
            nc.vector.tensor_scalar_add(den[:], sq[:], 1.0)
            rcp = ew("rcp")
            nc.vector.reciprocal(rcp[:], den[:])
            t2 = ew("t2")
            nc.vector.tensor_mul(t2[:], b_[:], z2r_sb[:])
            t3 = ew("t3")
            nc.vector.tensor_mul(t3[:], c_[:], rcp[:])
            s1 = ew("s1")
            nc.vector.tensor_add(s1[:], P0, t2[:])
            z2n_sb = ew("z2n_sb")
            nc.vector.tensor_add(z2n_sb[:], s1[:], t3[:])
            nc.sync.dma_start(d_z2n.ap(), z2n_sb[:])

            t4 = ew("t4")
            nc.vector.tensor_mul(t4[:], c_[:], d_[:])
            t5 = ew("t5")
            nc.vector.tensor_mul(t5[:], t4[:], arg[:])
            t6 = ew("t6")
            nc.vector.tensor_mul(t6[:], t5[:], rcp[:])
            t7 = ew("t7")
            nc.vector.tensor_mul(t7[:], t6[:], rcp[:])
            inner = ew("inner")
            nc.vector.scalar_tensor_tensor(
                inner[:], t7[:], -2.0, b_[:], op0=Alu.mult, op1=Alu.add
            )
            lg = ew("lg")
            nc.scalar.activation(lg[:], inner[:], AF.Ln)

            lg2 = mk(ewp, [128, NCHUNK], f32, "lg2")
            nc.vector.tensor_add(lg2[:], lg[:, :, 0], lg[:, :, 1])
            psl = mk(pp, [128, 512], f32, "ps")
            nc.tensor.matmul(
                psl[:NCHUNK, :4], lg2[:], mask_sb[:], start=True, stop=True
            )
            ld_sb = mk(ewp, [NCHUNK, 4], f32, "ld_sb")
            nc.vector.tensor_copy(ld_sb[:], psl[:NCHUNK, :4])
            nc.sync.dma_start(d_ld.ap(), ld_sb[:])

    nc.compile()
    return nc


def _get_program():
    if "nc" not in _CACHE:
        _CACHE["nc"] = _build_program()
    return _CACHE["nc"]


def _host_inputs(inputs):
    import ml_dtypes

    bf16 = ml_dtypes.bfloat16
    f32 = np.float32

    x = np.asarray(inputs["x"], f32)
    cond = np.asarray(inputs["cond"], f32)

    wc1T = np.zeros((128, H), f32)
    wc1T[: 2 * COND] = inputs["w_c1"].T
    wc1T = wc1T.astype(bf16)
    # partition-major packing: [128, ...] with large contiguous per-partition
    # chunks so SBUF DMAs are few big descriptors per partition
    wc2T = np.ascontiguousarray(
        inputs["w_c2"].T.reshape(4, 128, H).transpose(1, 0, 2)
    ).astype(bf16)
    # conv0 h-half collapsed through the rank-2 embedding; rows (k*2+d)
    w0 = np.asarray(inputs["w_conv0"], f32)
    we2 = np.asarray(inputs["w_embed"], f32)[:, :2]
    wh6 = np.zeros((128, H), f32)
    wh6[:6] = np.einsum("ock,cd->kdo", w0[:, :H, :], we2).reshape(6, H)
    wh6 = wh6.astype(bf16)
    # conv0 cond-half collapsed to per-sample vectors (types: sum_k, k0, k2)
    S = w0[:, H:, :]
    mats = np.stack([(S[:, :, 0] + S[:, :, 1] + S[:, :, 2]).T, S[:, :, 0].T, S[:, :, 2].T])
    qw = np.ascontiguousarray(
        mats.reshape(3, 4, 128, H).transpose(2, 1, 0, 3)
    ).astype(bf16)
    # b_embed folded through conv0 (rank-1 terms per type)
    be = np.asarray(inputs["b_embed"], f32)
    v = np.stack([w0[:, :H, k] @ be for k in range(3)])  # [3, H(out)]
    vb = np.ascontiguousarray(
        np.stack([v[0] + v[1] + v[2], v[0], v[2]])[None, :, :]
    ).astype(bf16)
    wcv = np.stack(
        [
            np.ascontiguousarray(
                np.transpose(inputs[f"w_conv{i}"], (1, 2, 0))
                .reshape(4, 128, 3, H)
                .transpose(1, 0, 2, 3)
            )
            for i in (1, 2, 3, 4)
        ]
    ).astype(bf16)
    woutT = np.ascontiguousarray(
        inputs["w_out"].T.reshape(4, 128, 10).transpose(1, 0, 2)
    ).astype(bf16)
    bias_all = np.stack(
        [inputs["b_embed"], inputs["b_c1"], inputs["b_c2"]]
        + [inputs[f"b_conv{i}"] for i in range(5)]
    ).astype(f32)
    bias_pack = np.ascontiguousarray(
        bias_all.reshape(8, 4, 128).transpose(2, 0, 1)
    ).astype(f32)
    brep = np.ascontiguousarray(
        np.broadcast_to(inputs["b_out"].astype(f32), (128, NCHUNK, 10))
    )
    mask = np.zeros((128, 4), f32)
    mask[np.arange(128), np.arange(128) // 32] = 1.0
    ik = np.zeros((128, WCOLS), f32)
    for s in range(NB):
        ik[s, ST * s : ST * s + T2] = 1.0
        ik[NB + s, ST * s] = -1.0
        ik[2 * NB + s, ST * s + T2 - 1] = -1.0
    ik = ik.astype(bf16)

    in_maps = []
    for c in range(NCORES):
        xs = x[c * NB : (c + 1) * NB]
        z1 = xs[:, :T2]
        z2 = xs[:, T2:]
        z1p = np.zeros((2, WCOLS + 2), np.float32)
        z1p[:, :WCOLS].reshape(2, NB, ST)[:, :, 1 : 1 + T2] = z1.transpose(2, 0, 1)
        z1g = np.zeros((32, WCOLS), np.float32)
        z1g[:6] = np.stack(
            [z1p[dd, k : k + WCOLS] for k in range(3) for dd in range(2)]
        )
        z1g = z1g.astype(bf16)
        z2r = np.ascontiguousarray(
            z2.reshape(NCHUNK, 4, T2, 2).transpose(1, 2, 0, 3)
        ).reshape(128, NCHUNK, 2)
        condT = np.zeros((128, NB), np.float32)
        condT[: 2 * COND] = cond[c * NB : (c + 1) * NB].reshape(NB, 2 * COND).T
        condT = condT.astype(bf16)
        in_maps.append(
            dict(
                z1g=z1g,
                z2r=z2r,
                condT=condT,
                wc1=wc1T,
                wc2=wc2T,
                wh6=wh6,
                qw=qw,
                vb=vb,
                ik=ik,
                wcv=wcv,
                wout=woutT,
                biases=bias_pack,
                brep=brep,
                mask=mask,
            )
        )
    return in_maps


def _assemble_output(x, results):
    z = np.empty((B, T, D), np.float32)
    ld = np.empty((B,), np.float32)
    for c in range(NCORES):
        z[c * NB : (c + 1) * NB, :T2] = x[c * NB : (c + 1) * NB, :T2]
        z2n = np.asarray(results[c]["z2n"], np.float32)
        z[c * NB : (c + 1) * NB, T2:] = (
            z2n.reshape(4, T2, NCHUNK, 2).transpose(2, 0, 1, 3).reshape(NB, T2, 2)
        )
        ld[c * NB : (c + 1) * NB] = np.asarray(results[c]["ld"], np.float32).reshape(
            NB
        )
    return z, ld


def run(inputs, trace=False, trace_cores=None):
    """Run on 8 NeuronCores; returns ((z, logdet), BassKernelResults)."""
    from concourse.bass_utils import run_bass_kernel_spmd

    nc = _get_program()
    in_maps = _host_inputs(inputs)
    res = run_bass_kernel_spmd(
        nc,
        in_maps,
        list(range(NCORES)),
        trace=trace,
        trace_cores=trace_cores if trace_cores is not None else list(range(NCORES)),
    )
    x = np.asarray(inputs["x"], np.float32)
    return _assemble_output(x, res.results), res


def kernel(**inputs):
    (z, ld), _ = run(inputs, trace=False)
    return z, ld


if __name__ == "__main__":
    print("build only:", _get_program())


# revision 53
# speedup vs baseline: 1.1329x; 1.0026x over previous
"""Trainium2 Bass kernel for nn_NlsqCond (ConvFlow NLSQ coupling layer).

Strategy: pure data parallel over batch B=256 -> 32 samples per core on 8
NeuronCores. Convs are computed as 3 shifted matmuls accumulating in PSUM
over a gap-column activation layout (34 columns per sample, zero guard
columns), so the k=3/pad=1 conv needs no boundary special-casing. Weights
are transposed host-side into lhsT layout and cast to bf16 (fp32 PSUM
accumulation); measured end-to-end error vs fp32 reference is ~1e-4.
The final projection is computed transposed ([cols, 10]) so the NLSQ
elementwise tail runs with full 128-partition parallelism; the per-sample
logdet partition-reduction is done with a small mask matmul.
"""

import math

import numpy as np

B, T, D, H, COND = 256, 64, 2, 512, 8
T2 = T // 2                      # 32
NCORES = 8
NB = B // NCORES                 # 32 samples per core
ST = T2 + 1                      # 33: stride; adjacent samples share one zero guard column
WCOLS = NB * ST + 1              # 1057 (one extra trailing guard column)
NG = 2                           # PSUM column groups per matmul set
GS = NB // NG                    # 16 samples per group
NCHUNK = NB * T2 // 128          # 8 column chunks of 128 for final proj
LOG_A = math.log(8.0 * math.sqrt(3.0) / 9.0 - 0.05)

_CACHE = {}


def _build_program():
    import concourse.bacc as bacc
    import concourse.mybir as mybir
    import concourse.tile as tile

    f32 = mybir.dt.float32
    bf16 = mybir.dt.bfloat16
    AF = mybir.ActivationFunctionType
    Alu = mybir.AluOpType

    nc = bacc.Bacc("TRN2", target_bir_lowering=False, debug=False)

    # ---- DRAM I/O ----
    # z1 in gap layout (zero guard columns), rows replicated for the 3 conv
    # shifts: row (k*2+d) col m = z1_gap[d, m+k]. Feeds conv0's h-half as a
    # single K=6 matmul per window.
    d_z1g = nc.dram_tensor("z1g", [32, WCOLS], bf16, kind="ExternalInput")
    d_z2r = nc.dram_tensor("z2r", [128, NCHUNK, 2], f32, kind="ExternalInput")
    d_condT = nc.dram_tensor("condT", [128, NB], bf16, kind="ExternalInput")
    d_wc1 = nc.dram_tensor("wc1", [128, H], bf16, kind="ExternalInput")
    # conv/linear weights packed partition-major so each DMA moves one large
    # contiguous chunk per partition (descriptor-rate, not bandwidth, limits
    # small-row DMAs)
    d_wc2 = nc.dram_tensor("wc2", [128, 4, H], bf16, kind="ExternalInput")
    # conv0 h-half collapsed to rank 6: wh6[k*2+d] = (w_conv0[:, :H, k] @ we).T
    d_wh6 = nc.dram_tensor("wh6", [128, H], bf16, kind="ExternalInput")
    # conv0 cond-half collapsed to per-sample vectors: types (sum_k, k=0, k=2)
    d_qw = nc.dram_tensor("qw", [128, 4, 3, H], bf16, kind="ExternalInput")
    # b_embed fold rank-1 terms per type
    d_vb = nc.dram_tensor("vb", [1, 3, H], bf16, kind="ExternalInput")
    # signed indicator matrix applying q to the conv0 windows on the PE:
    # row ty*32+s, col m -> coefficient of q[ty][s] in output position m+1
    d_ik = nc.dram_tensor("ik", [128, WCOLS], bf16, kind="ExternalInput")
    d_wcv = nc.dram_tensor("wcv", [4, 128, 4, 3, H], bf16, kind="ExternalInput")
    d_wout = nc.dram_tensor("wout", [128, 4, 10], bf16, kind="ExternalInput")
    d_bias = nc.dram_tensor("biases", [128, 8, 4], f32, kind="ExternalInput")
    d_brep = nc.dram_tensor("brep", [128, NCHUNK, 10], f32, kind="ExternalInput")
    d_mask = nc.dram_tensor("mask", [128, 4], f32, kind="ExternalInput")
    d_z2n = nc.dram_tensor("z2n", [128, NCHUNK, 2], f32, kind="ExternalOutput")
    d_ld = nc.dram_tensor("ld", [NCHUNK, 4], f32, kind="ExternalOutput")

    with tile.TileContext(nc) as tc:
        with (
            tc.tile_pool(name="w", bufs=1) as wp,
            tc.tile_pool(name="act", bufs=1) as ap_,
            tc.tile_pool(name="ew", bufs=1) as ewp,
            tc.tile_pool(name="ps", bufs=8, space="PSUM") as pp,
        ):
            def mk(pool, shape, dtype, tag):
                return pool.tile(shape, dtype, tag=tag, name=tag)

            # ---- SBUF loads ----
            # issue DMAs from several engine queues in parallel (each
            # DMA_DIRECT2D costs ~0.6-1.3us of serial issue time per queue)
            z1g_sb = mk(ap_, [128, WCOLS], bf16, "z1g")
            nc.gpsimd.dma_start(z1g_sb[:32], d_z1g.ap())
            for pb in (32, 64, 96):
                nc.vector.memset(z1g_sb[pb : pb + 32, :], 0.0)
            wh6_sb = mk(wp, [128, H], bf16, "wh6")
            nc.gpsimd.dma_start(wh6_sb[:], d_wh6.ap())
            condT_sb = mk(ap_, [128, NB], bf16, "condT")
            nc.gpsimd.dma_start(condT_sb[:], d_condT.ap())
            wc1_sb = mk(wp, [128, H], bf16, "wc1")
            nc.gpsimd.dma_start(wc1_sb[:], d_wc1.ap())
            bias_sb = mk(wp, [128, 8, 4], f32, "bias")
            nc.gpsimd.dma_start(bias_sb[:], d_bias.ap())
            wc2_sb = mk(wp, [128, 4, H], bf16, "wc2")
            nc.sync.dma_start(wc2_sb[:], d_wc2.ap())
            vb_sb = mk(wp, [1, 3, H], bf16, "vb")
            nc.sync.dma_start(vb_sb[:], d_vb.ap())
            qw_sb = mk(wp, [128, 4, 3, H], bf16, "qw")
            for kt in range(4):
                nc.sync.dma_start(qw_sb[:, kt], d_qw.ap()[:, kt])
            ik_sb = mk(wp, [128, WCOLS], bf16, "ik")
            nc.sync.dma_start(ik_sb[:96], d_ik.ap()[:96])
            nc.vector.memset(ik_sb[96:128, :], 0.0)
            wcv_sb = [mk(wp, [128, 4, 3, H], bf16, f"wcv_{l}") for l in range(4)]
            for l in range(4):
                nc.sync.dma_start(wcv_sb[l][:, 0:2], d_wcv.ap()[l, :, 0:2])
                nc.sync.dma_start(wcv_sb[l][:, 2:4], d_wcv.ap()[l, :, 2:4])
            wout_sb = mk(wp, [128, 4, 10], bf16, "wout")
            nc.sync.dma_start(wout_sb[:], d_wout.ap())
            brep_sb = mk(wp, [128, NCHUNK, 10], f32, "brep")
            nc.sync.dma_start(brep_sb[:], d_brep.ap())
            mask_sb = mk(wp, [128, 4], f32, "mask")
            nc.sync.dma_start(mask_sb[:], d_mask.ap())
            z2r_sb = mk(ewp, [128, NCHUNK, 2], f32, "z2r")
            nc.sync.dma_start(z2r_sb[:], d_z2r.ap())

            def bias_ap(idx, mt):
                return bias_sb[:, idx, mt : mt + 1]

            # ---- PE warm-up scratch (memset first so the PE can spin ASAP) ----
            wu_l = mk(ap_, [128, 128], bf16, "wu_l")
            wu_r = mk(ap_, [128, 512], bf16, "wu_r")
            nc.vector.memset(wu_l[:], 0.0)
            nc.vector.memset(wu_r[:], 0.0)

            # ---- activation ping-pong buffers (gap layout) ----
            # Only the gap guard columns need zeroing; valid columns are
            # always written before they are read.
            actB = [mk(ap_, [128, WCOLS], bf16, f"B{i}") for i in range(4)]
            actC = [mk(ap_, [128, WCOLS], bf16, f"C{i}") for i in range(4)]
            for ti, t_ in enumerate(actB + actC):
                v = t_[:, : NB * ST].rearrange("p (s c) -> p s c", c=ST)
                eng = nc.vector if ti % 2 else nc.gpsimd
                eng.memset(v[:, :, 0:1], 0.0)
                eng.memset(t_[:, NB * ST :], 0.0)
            ones_sb = mk(ap_, [1, NB], bf16, "ones")
            nc.vector.memset(ones_sb[:], 1.0)

            def valid(tl, g=None):
                v = tl[:, : NB * ST].rearrange("p (s c) -> p s c", c=ST)
                if g is None:
                    return v[:, :, 1 : 1 + T2]
                return v[:, g * GS : (g + 1) * GS, 1 : 1 + T2]

            # ---- PE warm-up spin ----
            # The PE HAM clock gate starts at 1.2 GHz and only releases to
            # 2.4 GHz after ~3.4us of sustained activity. Matmul on scratch
            # zeros while the weight DMAs stream in, so the real conv stack
            # runs warm from its first instruction.
            ps_w = mk(pp, [128, 512], f32, "ps")
            for i in range(10):
                nc.tensor.matmul(
                    ps_w[:], wu_l[:], wu_r[:], start=(i == 0), stop=(i == 9)
                )

            # preload the ACT transcendental tables so the elementwise tail
            # doesn't pay the table-swap latency
            scr = mk(ewp, [1, 4], f32, "scr")
            nc.scalar.activation(scr[:, 0:1], wu_l[:1, 0:1], AF.Exp)
            nc.scalar.activation(scr[:, 1:2], wu_l[:1, 0:1], AF.Tanh)
            nc.scalar.activation(scr[:, 2:3], wu_l[:1, 0:1], AF.Ln, bias=1.0)

            # ---- cond MLP: c2 = relu(W2 relu(W1 c + b1) + b2) ----
            c1_sb = [mk(ap_, [128, NB], bf16, f"c1_{i}") for i in range(4)]
            c2_sb = [mk(ap_, [128, NB], bf16, f"c2_{i}") for i in range(4)]
            for mt in range(4):
                ps = mk(pp, [128, 512], f32, "ps")
                nc.tensor.matmul(
                    ps[:, :NB],
                    wc1_sb[:, mt * 128 : (mt + 1) * 128],
                    condT_sb[:],
                    start=True,
                    stop=True,
                )
                nc.scalar.activation(
                    c1_sb[mt][:], ps[:, :NB], AF.Relu, bias=bias_ap(1, mt)
                )
            for mt in range(4):
                ps = mk(pp, [128, 512], f32, "ps")
                for kt in range(4):
                    nc.tensor.matmul(
                        ps[:, :NB],
                        wc2_sb[:, kt, mt * 128 : (mt + 1) * 128],
                        c1_sb[kt][:],
                        start=(kt == 0),
                        stop=(kt == 3),
                    )
                nc.scalar.activation(
                    c2_sb[mt][:], ps[:, :NB], AF.Relu, bias=bias_ap(2, mt)
                )

            WINDOWS = [(0, 15), (15, 15), (30, 2)]  # (sample base, n samples)

            def relu_copy(L, mt, wi, pv_, dsts, sb, ns):
                dv_ = dsts[mt][:, : NB * ST].rearrange("p (s c) -> p s c", c=ST)[
                    :, sb : sb + ns, 1 : 1 + T2
                ]
                # relu copies on DVE only: any ACT Relu evicts the
                # transcendental tables and the elementwise tail would pay
                # two ~1.3us table reloads
                nc.vector.tensor_scalar(
                    dv_, pv_, bias_ap(3 + L, mt), 0.0, Alu.add, Alu.max
                )

            # ---- conv0 ----
            # qT[ty*32+s, o]: per-sample cond/bias vectors, computed
            # transposed so they can be applied to the conv0 windows by a
            # single K=96 indicator matmul inside the PSUM accumulation:
            #   ty=0: (sum_k W0k_c) @ c2[s] + sum_k(W0k_h @ b_embed)
            #   ty=1: W00_c @ c2[s] + v0   (subtracted at t=0)
            #   ty=2: W02_c @ c2[s] + v2   (subtracted at t=31)
            psq = mk(pp, [96, 512], f32, "ps")
            for ty in range(3):
                for kt in range(4):
                    nc.tensor.matmul(
                        psq[ty * NB : (ty + 1) * NB, :],
                        c2_sb[kt][:],
                        qw_sb[:, kt, ty, :],
                        start=(kt == 0),
                        stop=False,
                    )
                nc.tensor.matmul(
                    psq[ty * NB : (ty + 1) * NB, :],
                    ones_sb[:],
                    vb_sb[:, ty, :],
                    start=False,
                    stop=True,
                )
            qT_sb = mk(ap_, [128, H], bf16, "qT")
            nc.vector.memset(qT_sb[96:128, :], 0.0)
            nc.vector.tensor_copy(qT_sb[:96, :], psq[:])

            # h-half: single K=6 matmul per window (the 3 conv shifts are
            # baked into the replicated z1g rows); q applied via indicator
            for mt in range(4):
                pss = [mk(pp, [128, 512], f32, "ps") for _ in WINDOWS]
                for wi, (sb, ns) in enumerate(WINDOWS):
                    n = ns * ST - 1
                    base = sb * ST
                    nc.tensor.matmul(
                        pss[wi][:, :n],
                        wh6_sb[:, mt * 128 : (mt + 1) * 128],
                        z1g_sb[:, base : base + n],
                        start=True,
                        stop=False,
                    )
                    nc.tensor.matmul(
                        pss[wi][:, :n],
                        qT_sb[:, mt * 128 : (mt + 1) * 128],
                        ik_sb[:, base : base + n],
                        start=False,
                        stop=True,
                    )
                for wi, (sb, ns) in enumerate(WINDOWS):
                    pv3 = pss[wi][:, : ns * ST].rearrange("p (s c) -> p s c", c=ST)
                    relu_copy(0, mt, wi, pv3[:, :, 0:T2], actB, sb, ns)

            # ---- conv1..4 ----
            # Matmul moving operands must be single-free-dim, so each conv
            # matmul streams a contiguous window of the gap layout; outputs
            # at gap positions are garbage and simply never read back.
            for L in range(1, 5):
                srcs = [actB, actC, actB, actC][L - 1]
                wts = wcv_sb[L - 1][:]
                dsts = actB if L % 2 == 0 else actC
                for mt in range(4):
                    pss = [mk(pp, [128, 512], f32, "ps") for _ in WINDOWS]
                    nacc = 12
                    i = 0
                    # kt outer: matches weight-DMA arrival order
                    for kt in range(4):
                        for k in range(3):
                            lhsT = wts[:, kt, k, mt * 128 : (mt + 1) * 128]
                            for wi, (sb, ns) in enumerate(WINDOWS):
                                n = ns * ST - 1
                                base = sb * ST + k
                                nc.tensor.matmul(
                                    pss[wi][:, :n],
                                    lhsT,
                                    srcs[kt][:, base : base + n],
                                    start=(i == 0),
                                    stop=(i == nacc - 1),
                                )
                            i += 1
                    for wi, (sb, ns) in enumerate(WINDOWS):
                        pv3 = pss[wi][:, : ns * ST].rearrange(
                            "p (s c) -> p s c", c=ST
                        )
                        relu_copy(L, mt, wi, pv3[:, :, 0:T2], dsts, sb, ns)
            srcs = actB

            # ---- compact the final activation (drop gap columns) ----
            h5 = [mk(ap_, [128, NB * T2], bf16, f"h5_{i}") for i in range(4)]
            for kt in range(4):
                eng = nc.scalar if kt % 2 == 0 else nc.vector
                if kt % 2 == 0:
                    nc.scalar.copy(
                        h5[kt][:].rearrange("p (s t) -> p s t", t=T2),
                        valid(srcs[kt]),
                    )
                else:
                    nc.vector.tensor_copy(
                        h5[kt][:].rearrange("p (s t) -> p s t", t=T2),
                        valid(srcs[kt]),
                    )

            # ---- final projection (transposed): out[col, 10] ----
            pso = mk(pp, [128, 512], f32, "ps")
            for j in range(NCHUNK):
                for kt in range(4):
                    nc.tensor.matmul(
                        pso[:, j * 10 : (j + 1) * 10],
                        h5[kt][:, j * 128 : (j + 1) * 128],
                        wout_sb[:, kt],
                        start=(kt == 0),
                        stop=(kt == 3),
                    )

            # ---- NLSQ elementwise tail ----
            def ew(tag):
                return mk(ewp, [128, NCHUNK, 2], f32, tag)

            params = mk(ewp, [128, NCHUNK, 10], f32, "params")
            nc.vector.tensor_add(
                params[:],
                pso[:, : NCHUNK * 10].rearrange("p (j q) -> p j q", q=10),
                brep_sb[:],
            )
            pv = params[:].rearrange("p j (a q) -> p j a q", q=5)
            P0, P1, P2, P3, P4 = (pv[:, :, :, i] for i in range(5))

            loga_sb = mk(ewp, [128, 1], f32, "loga")
            nc.vector.memset(loga_sb[:], LOG_A)

            # group ACT functions (Exp x3, then Tanh, Ln last) to minimize
            # activation-table reloads
            u = ew("u")
            nc.vector.tensor_tensor(u[:], P1, P3, op=Alu.subtract)
            b_ = ew("b_")
            nc.scalar.activation(b_[:], P1, AF.Exp, scale=0.4)
            d_ = ew("d_")
            nc.scalar.activation(d_[:], P3, AF.Exp, scale=0.4)
            e = ew("e")
            nc.scalar.activation(e[:], u[:], AF.Exp, scale=0.4, bias=loga_sb[:])
            th = ew("th")
            nc.scalar.activation(th[:], P2, AF.Tanh, scale=0.3)
            c_ = ew("c_")
            nc.vector.tensor_mul(c_[:], th[:], e[:])
            t1 = ew("t1")
            nc.vector.tensor_mul(t1[:], d_[:], z2r_sb[:])
            arg = ew("arg")
            nc.vector.tensor_add(arg[:], t1[:], P4)
            sq = ew("sq")
            nc.vector.tensor_mul(sq[:], arg[:], arg[:])
            den = ew("den")
            nc.vector.tensor_scalar_add(den[:], sq[:], 1.0)
            rcp = ew("rcp")
            nc.vector.reciprocal(rcp[:], den[:])
            t2 = ew("t2")
            nc.vector.tensor_mul(t2[:], b_[:], z2r_sb[:])
            t3 = ew("t3")
            nc.vector.tensor_mul(t3[:], c_[:], rcp[:])
            s1 = ew("s1")
            nc.vector.tensor_add(s1[:], P0, t2[:])
            z2n_sb = ew("z2n_sb")
            nc.vector.tensor_add(z2n_sb[:], s1[:], t3[:])
            nc.sync.dma_start(d_z2n.ap(), z2n_sb[:])

            t4 = ew("t4")
            nc.vector.tensor_mul(t4[:], c_[:], d_[:])
            t5 = ew("t5")
            nc.vector.tensor_mul(t5[:], t4[:], arg[:])
            t6 = ew("t6")
            nc.vector.tensor_mul(t6[:], t5[:], rcp[:])
            t7 = ew("t7")
            nc.vector.tensor_mul(t7[:], t6[:], rcp[:])
            inner = ew("inner")
            nc.vector.scalar_tensor_tensor(
                inner[:], t7[:], -2.0, b_[:], op0=Alu.mult, op1=Alu.add
            )
            lg = ew("lg")
            nc.scalar.activation(lg[:], inner[:], AF.Ln)

            lg2 = mk(ewp, [128, NCHUNK], f32, "lg2")
            nc.vector.tensor_add(lg2[:], lg[:, :, 0], lg[:, :, 1])
            psl = mk(pp, [128, 512], f32, "ps")
            nc.tensor.matmul(
                psl[:NCHUNK, :4], lg2[:], mask_sb[:], start=True, stop=True
            )
            ld_sb = mk(ewp, [NCHUNK, 4], f32, "ld_sb")
            nc.vector.tensor_copy(ld_sb[:], psl[:NCHUNK, :4])
            nc.sync.dma_start(d_ld.ap(), ld_sb[:])

    nc.compile()
    return nc


def _get_program():
    if "nc" not in _CACHE:
        _CACHE["nc"] = _build_program()
    return _CACHE["nc"]


def _host_inputs(inputs):
    import ml_dtypes

    bf16 = ml_dtypes.bfloat16
    f32 = np.float32

    x = np.asarray(inputs["x"], f32)
    cond = np.asarray(inputs["cond"], f32)

    wc1T = np.zeros((128, H), f32)
    wc1T[: 2 * COND] = inputs["w_c1"].T
    wc1T = wc1T.astype(bf16)
    # partition-major packing: [128, ...] with large contiguous per-partition
    # chunks so SBUF DMAs are few big descriptors per partition
    wc2T = np.ascontiguousarray(
        inputs["w_c2"].T.reshape(4, 128, H).transpose(1, 0, 2)
    ).astype(bf16)
    # conv0 h-half collapsed through the rank-2 embedding; rows (k*2+d)
    w0 = np.asarray(inputs["w_conv0"], f32)
    we2 = np.asarray(inputs["w_embed"], f32)[:, :2]
    wh6 = np.zeros((128, H), f32)
    wh6[:6] = np.einsum("ock,cd->kdo", w0[:, :H, :], we2).reshape(6, H)
    wh6 = wh6.astype(bf16)
    # conv0 cond-half collapsed to per-sample vectors (types: sum_k, k0, k2)
    S = w0[:, H:, :]
    mats = np.stack([(S[:, :, 0] + S[:, :, 1] + S[:, :, 2]).T, S[:, :, 0].T, S[:, :, 2].T])
    qw = np.ascontiguousarray(
        mats.reshape(3, 4, 128, H).transpose(2, 1, 0, 3)
    ).astype(bf16)
    # b_embed folded through conv0 (rank-1 terms per type)
    be = np.asarray(inputs["b_embed"], f32)
    v = np.stack([w0[:, :H, k] @ be for k in range(3)])  # [3, H(out)]
    vb = np.ascontiguousarray(
        np.stack([v[0] + v[1] + v[2], v[0], v[2]])[None, :, :]
    ).astype(bf16)
    wcv = np.stack(
        [
            np.ascontiguousarray(
                np.transpose(inputs[f"w_conv{i}"], (1, 2, 0))
                .reshape(4, 128, 3, H)
                .transpose(1, 0, 2, 3)
            )
            for i in (1, 2, 3, 4)
        ]
    ).astype(bf16)
    woutT = np.ascontiguousarray(
        inputs["w_out"].T.reshape(4, 128, 10).transpose(1, 0, 2)
    ).astype(bf16)
    bias_all = np.stack(
        [inputs["b_embed"], inputs["b_c1"], inputs["b_c2"]]
        + [inputs[f"b_conv{i}"] for i in range(5)]
    ).astype(f32)
    bias_pack = np.ascontiguousarray(
        bias_all.reshape(8, 4, 128).transpose(2, 0, 1)
    ).astype(f32)
    brep = np.ascontiguousarray(
        np.broadcast_to(inputs["b_out"].astype(f32), (128, NCHUNK, 10))
    )
    mask = np.zeros((128, 4), f32)
    mask[np.arange(128), np.arange(128) // 32] = 1.0
    ik = np.zeros((128, WCOLS), f32)
    for s in range(NB):
        ik[s, ST * s : ST * s + T2] = 1.0
        ik[NB + s, ST * s] = -1.0
        ik[2 * NB + s, ST * s + T2 - 1] = -1.0
    ik = ik.astype(bf16)

    in_maps = []
    for c in range(NCORES):
        xs = x[c * NB : (c + 1) * NB]
        z1 = xs[:, :T2]
        z2 = xs[:, T2:]
        z1p = np.zeros((2, WCOLS + 2), np.float32)
        z1p[:, : NB * ST].reshape(2, NB, ST)[:, :, 1 : 1 + T2] = z1.transpose(2, 0, 1)
        z1g = np.zeros((32, WCOLS), np.float32)
        z1g[:6] = np.stack(
            [z1p[dd, k : k + WCOLS] for k in range(3) for dd in range(2)]
        )
        z1g = z1g.astype(bf16)
        z2r = np.ascontiguousarray(
            z2.reshape(NCHUNK, 4, T2, 2).transpose(1, 2, 0, 3)
        ).reshape(128, NCHUNK, 2)
        condT = np.zeros((128, NB), np.float32)
        condT[: 2 * COND] = cond[c * NB : (c + 1) * NB].reshape(NB, 2 * COND).T
        condT = condT.astype(bf16)
        in_maps.append(
            dict(
                z1g=z1g,
                z2r=z2r,
                condT=condT,
                wc1=wc1T,
                wc2=wc2T,
                wh6=wh6,
                qw=qw,
                vb=vb,
                ik=ik,
                wcv=wcv,
                wout=woutT,
                biases=bias_pack,
                brep=brep,
                mask=mask,
            )
        )
    return in_maps


def _assemble_output(x, results):
    z = np.empty((B, T, D), np.float32)
    ld = np.empty((B,), np.float32)
    for c in range(NCORES):
        z[c * NB : (c + 1) * NB, :T2] = x[c * NB : (c + 1) * NB, :T2]
        z2n = np.asarray(results[c]["z2n"], np.float32)
        z[c * NB : (c + 1) * NB, T2:] = (
            z2n.reshape(4, T2, NCHUNK, 2).transpose(2, 0, 1, 3).reshape(NB, T2, 2)
        )
        ld[c * NB : (c + 1) * NB] = np.asarray(results[c]["ld"], np.float32).reshape(
            NB
        )
    return z, ld


def run(inputs, trace=False, trace_cores=None):
    """Run on 8 NeuronCores; returns ((z, logdet), BassKernelResults)."""
    from concourse.bass_utils import run_bass_kernel_spmd

    nc = _get_program()
    in_maps = _host_inputs(inputs)
    res = run_bass_kernel_spmd(
        nc,
        in_maps,
        list(range(NCORES)),
        trace=trace,
        trace_cores=trace_cores if trace_cores is not None else list(range(NCORES)),
    )
    x = np.asarray(inputs["x"], np.float32)
    return _assemble_output(x, res.results), res


def kernel(**inputs):
    (z, ld), _ = run(inputs, trace=False)
    return z, ld


if __name__ == "__main__":
    print("build only:", _get_program())
